# revision 1
# baseline (speedup 1.0000x reference)
"""Trainium2 Bass kernel for the MHA-with-diagonal-softmax module.

Computation (per batch b):
    q = rope(x @ Wq.T), k = rope(x @ Wk.T), v = x @ Wv.T      (per head, DH=128)
    sumexp[s,h] = sum_k exp(q_h[s] . k_h[k] * DH^-0.5)
    diag[s,h]   = q_h[s] . k_h[s] * DH^-0.5
    w = exp(diag) / sumexp
    out = (w * v) @ Wo.T

Sharding: 8 cores = 2 (batch) x 4 (head groups of 4 heads).
Each core computes q/k/v for its 4 heads in transposed [head_dim, seq]
layout, the per-position softmax-diagonal weights, and a partial output
projection (its heads' rows of Wo), written as 2 head-pair partials that
the host sums.

On-chip dtype is fp16 (same PE throughput as bf16, 8x lower rounding
error - matters because exp() amplifies absolute score error), with fp32
PSUM accumulation everywhere.
"""

import numpy as np
from contextlib import ExitStack

# Problem constants (hardcoded per harness contract).
B, S, D, H, DH = 2, 2048, 2048, 16, 128
HPC = 4            # heads per core
NHL = HPC * DH     # 512 local head dims per core
KB = D // 128      # 16 contraction blocks
SB = S // 128      # 16 seq blocks of 128
SC = S // 512      # 4 seq/emb chunks of 512
NCORES = 8

_CACHE = {}


def _build_nc():
    import concourse.bass as bass
    import concourse.tile as tile
    from concourse import bacc, mybir
    from concourse.masks import make_identity

    F16 = mybir.dt.float16
    F32 = mybir.dt.float32
    AF = mybir.ActivationFunctionType
    ALU = mybir.AluOpType
    AX = mybir.AxisListType

    # Bacc (not raw Bass): its compile() splits multi-sem waits into
    # event-semaphore instructions - HW allows at most 1 wait per inst.
    nc = bacc.Bacc("TRN2", target_bir_lowering=False, debug=False)

    xT = nc.dram_tensor("xT", [D, S], F16, kind="ExternalInput").ap()
    wq = nc.dram_tensor("wq", [D, NHL], F16, kind="ExternalInput").ap()
    wk = nc.dram_tensor("wk", [D, NHL], F16, kind="ExternalInput").ap()
    wv = nc.dram_tensor("wv", [D, NHL], F16, kind="ExternalInput").ap()
    wo = nc.dram_tensor("wo", [NHL, D], F16, kind="ExternalInput").ap()
    ropeA = nc.dram_tensor("ropeA", [128, S], F16, kind="ExternalInput").ap()
    ropeB = nc.dram_tensor("ropeB", [128, S], F16, kind="ExternalInput").ap()
    y = nc.dram_tensor("y", [2, S, D], F16, kind="ExternalOutput").ap()

    xT_r = xT.rearrange("(a p) s -> a p s", p=128)
    wq_r = wq.rearrange("(a p) m -> a p m", p=128)
    wk_r = wk.rearrange("(a p) m -> a p m", p=128)
    wv_r = wv.rearrange("(a p) m -> a p m", p=128)
    wo_r = wo.rearrange("(h p) n -> h p n", p=128)

    with tile.TileContext(nc) as tc, ExitStack() as ctx:
        pool = ctx.enter_context(tc.tile_pool(name="sb", bufs=1))
        pp = ctx.enter_context(tc.tile_pool(name="ps", bufs=1, space="PSUM"))

        # ---- constants ----
        ra = pool.tile([128, S], F16, name="ra")
        rb = pool.tile([128, S], F16, name="rb")
        # SWDGE: a wide HWDGE DMA fans out over several HW queues, and a
        # DVE/ACT consumer then needs one sync-wait per queue, exceeding
        # the instruction's wait-slot budget at compile time.
        nc.gpsimd.dma_start(ra[:, :], ropeA[:, :])
        nc.gpsimd.dma_start(rb[:, :], ropeB[:, :])
        ident = pool.tile([128, 128], F32, name="ident")
        make_identity(nc, ident[:, :])
        onesf = pool.tile([128, 128], F32, name="onesf")
        nc.gpsimd.memset(onesf[:, :], 1.0)
        ones1 = pool.tile([128, 128], F16, name="ones1")
        nc.gpsimd.memset(ones1[:, :], 1.0)

        # ---- x resident in SBUF ----
        xsb = pool.tile([128, KB, S], F16, name="xsb")
        for kb in range(KB):
            nc.sync.dma_start(xsb[:, kb, :], xT_r[kb])

        # ---- persistent q/k/v head tiles ([head_dim, seq] layout) ----
        qh = [pool.tile([128, S], F16, name=f"qh{h}") for h in range(HPC)]
        kh = [pool.tile([128, S], F16, name=f"kh{h}") for h in range(HPC)]
        vh = [pool.tile([128, S], F16, name=f"vh{h}") for h in range(HPC)]

        # per-head row vectors live at partition 32*h (engine ops only
        # support start partitions that are multiples of 32)
        ds_diag = pool.tile([128, S], F32, name="ds_diag")
        ds_sum = pool.tile([128, S], F16, name="ds_sum")
        w4 = pool.tile([128, S], F16, name="w4")
        sumf = [pool.tile([128, SB], F32, name=f"sumf{h}") for h in range(HPC)]

        def load_w(src_r, nblk, tag="w"):
            t = pool.tile([128, nblk, 512 * (KB // nblk)], F16, name="wt",
                          tag=tag, bufs=2)
            for i in range(nblk):
                nc.sync.dma_start(t[:, i, :], src_r[i])
            return t

        def proj_chunk(wt, dests, mt, sc):
            # dests[mt][:, sc-chunk] <- (wt[:, :, mt] block).T @ x chunk
            ps = pp.tile([128, 512], F32, name="mmps", tag="mm", bufs=2)
            for kb in range(KB):
                nc.tensor.matmul(
                    ps[:, :],
                    wt[:, kb, mt * 128:(mt + 1) * 128],
                    xsb[:, kb, sc * 512:(sc + 1) * 512],
                    start=(kb == 0), stop=(kb == KB - 1))
            nc.scalar.activation(
                dests[mt][:, sc * 512:(sc + 1) * 512], ps[:, :], AF.Copy)

        def proj(wt, dests):
            for mt in range(HPC):
                for sc in range(SC):
                    proj_chunk(wt, dests, mt, sc)

        def rope(dst):
            # dst (in place): top = te*cos - to*sin ; bottom = te*sin + to*cos
            # ra = [cosT; cosT], rb = [-sinT; sinT]; swap = halves exchanged.
            for c in range(2):
                sl = slice(c * 1024, (c + 1) * 1024)
                # SWDGE (gpsimd) keeps this 1 queue -> 1 sem; a wide HWDGE
                # sbuf->sbuf DMA fans out over many queues and blows the
                # consumer's sync-wait slot budget.
                swp = pool.tile([128, 1024], F16, name="swp", tag="swp", bufs=1)
                nc.gpsimd.dma_start(swp[0:64, :], dst[64:128, sl])
                nc.gpsimd.dma_start(swp[64:128, :], dst[0:64, sl])
                u = pool.tile([128, 1024], F16, name="u", tag="sc", bufs=2)
                nc.vector.tensor_mul(u[:, :], dst[:, sl], ra[:, sl])
                v2 = pool.tile([128, 1024], F16, name="v2", tag="sc", bufs=2)
                nc.vector.tensor_mul(v2[:, :], swp[:, :], rb[:, sl])
                nc.vector.tensor_add(dst[:, sl], u[:, :], v2[:, :])

        def diag(h):
            # ds_diag[32h, s] = sum_m qh[h][m, s] * kh[h][m, s]  (fp32)
            hp = 32 * h
            for c in range(2):
                sl = slice(c * 1024, (c + 1) * 1024)
                pr = pool.tile([128, 1024], F32, name="pr", tag="pr", bufs=1)
                nc.vector.tensor_mul(pr[:, :], qh[h][:, sl], kh[h][:, sl])
                for cc in range(2):
                    dps = pp.tile([128, 512], F32, name="dps", tag="sm", bufs=2)
                    nc.tensor.matmul(dps[:, :], onesf[:, :],
                                     pr[:, cc * 512:(cc + 1) * 512],
                                     start=True, stop=True)
                    o = (2 * c + cc) * 512
                    nc.scalar.activation(ds_diag[hp:hp + 1, o:o + 512],
                                         dps[hp:hp + 1, :], AF.Copy)

        # ====== phase 2 pieces ======
        def scores_sq(h, sq, fillers):
            """One query block: 4 score MMs into a 4-bank psum tile, one wide
            exp with fused row-sum. `fillers` = list of callables emitting
            independent PE work, interleaved so the in-order PE queue always
            has runnable matmuls while ACT drains the exp (keeps HAM warm)."""
            sps = pp.tile([128, S], F32, name="sps", tag="sco", bufs=1)
            for ck in range(SC):
                nc.tensor.matmul(sps[:, ck * 512:(ck + 1) * 512],
                                 qh[h][:, sq * 128:(sq + 1) * 128],
                                 kh[h][:, ck * 512:(ck + 1) * 512],
                                 start=True, stop=True)
            for f in fillers:
                f()
            ex = pool.tile([128, S], F16, name="ex", tag="ex", bufs=1)
            nc.scalar.activation(ex[:, :], sps[:, :], AF.Exp,
                                 accum_out=sumf[h][:, sq:sq + 1])

        def head_sum_tail(h):
            # recip -> transpose -> [1, S] row of ds_sum
            rs = pool.tile([128, SB], F32, name="rs", tag="rs", bufs=2)
            nc.vector.reciprocal(rs[:, :], sumf[h][:, :])
            tps = pp.tile([16, 128], F32, name="tps", tag="sm", bufs=2)
            nc.tensor.transpose(tps[:, :], rs[:, :], ident[:, :])
            st = pool.tile([16, 128], F16, name="st", tag="st", bufs=2)
            nc.vector.tensor_copy(st[:, :], tps[:, :])
            nc.gpsimd.dma_start(ds_sum[32 * h:32 * h + 1, :], st[:, :])

        def pair_head(p):
            # w = exp(diag) * recip(sumexp); attn = w (bcast) * v, into kh
            h0, h1 = 2 * p, 2 * p + 1
            expd = pool.tile([128, S], F16, name="expd", tag="expd", bufs=2)
            for h in (h0, h1):
                hp = 32 * h
                nc.scalar.activation(expd[hp:hp + 1, :], ds_diag[hp:hp + 1, :],
                                     AF.Exp)
                nc.vector.tensor_mul(w4[hp:hp + 1, :], expd[hp:hp + 1, :],
                                     ds_sum[hp:hp + 1, :])
                wb = pool.tile([128, S], F16, name="wb", tag="wb", bufs=1)
                for ck in range(SC):
                    bps = pp.tile([128, 512], F32, name="bps", tag="mm", bufs=2)
                    nc.tensor.matmul(bps[:, :], ones1[hp:hp + 1, :],
                                     w4[hp:hp + 1, ck * 512:(ck + 1) * 512],
                                     start=True, stop=True,
                                     tile_position=(hp, 0))
                    nc.vector.tensor_copy(wb[:, ck * 512:(ck + 1) * 512],
                                          bps[:, :])
                nc.vector.tensor_mul(kh[h][:, :], wb[:, :], vh[h][:, :])

        def oproj_group(p, sb, ncx):
            h0, h1 = 2 * p, 2 * p + 1
            ps = pp.tile([128, 512], F32, name="ops", tag="mm", bufs=2)
            for i, h in enumerate((h0, h1)):
                nc.tensor.matmul(
                    ps[:, :], kh[h][:, sb * 128:(sb + 1) * 128],
                    wot[:, h, ncx * 512:(ncx + 1) * 512],
                    start=(i == 0), stop=(i == 1))
            yt = pool.tile([128, 512], F16, name="yt", tag="yt", bufs=2)
            nc.vector.tensor_copy(yt[:, :], ps[:, :])
            nc.sync.dma_start(
                y[p, sb * 128:(sb + 1) * 128,
                  ncx * 512:(ncx + 1) * 512], yt[:, :])

        # ================= emission =================
        # dense PE phase: K and Q projections + rope + diag
        wkt = load_w(wk_r, KB)
        wqt = load_w(wq_r, KB)
        proj(wkt, kh)
        for h in range(HPC):
            rope(kh[h])
        proj(wqt, qh)
        for h in range(HPC):
            rope(qh[h])
            diag(h)
        # wv reuses wk's slot, wo reuses wq's slot (tag bufs=2)
        wvt = load_w(wv_r, KB)
        wot = load_w(wo_r, HPC)

        # scores streams, with independent matmul work as filler:
        #   head 0/1 slots <- V projection chunks (16 groups of 16 MMs)
        #   head 2 slots   <- pair-0 output projection (64 groups of 2 MMs)
        #   head 3 slots   <- none available (paced by exp)
        vfill = [(mt, sc) for mt in range(HPC) for sc in range(SC)]
        for sq in range(SB):
            f = []
            if sq % 2 == 0 and vfill:
                mt, sc = vfill.pop(0)
                f.append(lambda mt=mt, sc=sc: proj_chunk(wvt, vh, mt, sc))
            scores_sq(0, sq, f)
        head_sum_tail(0)
        for sq in range(SB):
            f = []
            if sq % 2 == 0 and vfill:
                mt, sc = vfill.pop(0)
                f.append(lambda mt=mt, sc=sc: proj_chunk(wvt, vh, mt, sc))
            scores_sq(1, sq, f)
        head_sum_tail(1)
        pair_head(0)
        ofill = [(sb, ncx) for sb in range(SB) for ncx in range(SC)]
        for sq in range(SB):
            f = []
            for _ in range(4):
                if ofill:
                    sb, ncx = ofill.pop(0)
                    f.append(lambda sb=sb, ncx=ncx: oproj_group(0, sb, ncx))
            scores_sq(2, sq, f)
        head_sum_tail(2)
        for sq in range(SB):
            f = []
            if ofill:
                sb, ncx = ofill.pop(0)
                f.append(lambda sb=sb, ncx=ncx: oproj_group(0, sb, ncx))
            scores_sq(3, sq, f)
        head_sum_tail(3)
        for sb, ncx in ofill:
            oproj_group(0, sb, ncx)
        pair_head(1)
        for sb in range(SB):
            for ncx in range(SC):
                oproj_group(1, sb, ncx)

    nc.compile()
    return nc


def _get_nc():
    if "nc" not in _CACHE:
        _CACHE["nc"] = _build_nc()
    return _CACHE["nc"]


_PERM = np.concatenate([np.arange(0, DH, 2), np.arange(1, DH, 2)])


def _host_inputs(x, rope_cos, rope_sin, Wq, Wk, Wv, Wo):
    """Build the 8 per-core input maps."""
    f16 = np.float16
    cosT = np.ascontiguousarray(np.asarray(rope_cos, np.float32)[0, :, 0, :].T)
    sinT = np.ascontiguousarray(np.asarray(rope_sin, np.float32)[0, :, 0, :].T)
    ra = np.concatenate([cosT, cosT], 0).astype(f16)
    rb = np.concatenate([-sinT, sinT], 0).astype(f16)

    Wq = np.asarray(Wq, np.float32)
    Wk = np.asarray(Wk, np.float32)
    Wv = np.asarray(Wv, np.float32)
    Wo = np.asarray(Wo, np.float32)
    x = np.asarray(x, np.float32)

    xTb = [np.ascontiguousarray(x[b].T).astype(f16) for b in range(B)]
    scale = DH ** -0.5

    in_maps = []
    for core in range(NCORES):
        b, g = divmod(core, HPC)
        hs = g * HPC
        rows = np.concatenate(
            [h * DH + _PERM for h in range(hs, hs + HPC)])      # deinterleave
        rows_v = np.arange(hs * DH, (hs + HPC) * DH)
        in_maps.append({
            "xT": xTb[b],
            "wq": np.ascontiguousarray((Wq[rows] * scale).T).astype(f16),
            "wk": np.ascontiguousarray(Wk[rows].T).astype(f16),
            "wv": np.ascontiguousarray(Wv[rows_v].T).astype(f16),
            "wo": np.ascontiguousarray(Wo[:, rows_v].T).astype(f16),
            "ropeA": ra,
            "ropeB": rb,
        })
    return in_maps


def kernel(x, rope_cos, rope_sin, Wq, Wk, Wv, Wo, _trace=False, _trace_cores=None):
    from concourse.bass_utils import run_bass_kernel_spmd

    nc = _get_nc()
    in_maps = _host_inputs(x, rope_cos, rope_sin, Wq, Wk, Wv, Wo)
    res = run_bass_kernel_spmd(nc, in_maps, list(range(NCORES)),
                               trace=_trace, trace_cores=_trace_cores)
    _CACHE["last_result"] = res

    out = np.zeros((B, S, D), np.float32)
    for core in range(NCORES):
        b = core // HPC
        out[b] += res.results[core]["y"].astype(np.float32).sum(axis=0)
    return out



# revision 8
# speedup vs baseline: 1.1995x; 1.1995x over previous
"""Trainium2 Bass kernel for the MHA-with-diagonal-softmax module.

Computation (per batch b):
    q = rope(x @ Wq.T), k = rope(x @ Wk.T), v = x @ Wv.T      (per head, DH=128)
    sumexp[s,h] = sum_k exp(q_h[s] . k_h[k] * DH^-0.5)
    diag[s,h]   = q_h[s] . k_h[s] * DH^-0.5
    w = exp(diag) / sumexp
    out = (w * v) @ Wo.T

Sharding: 8 cores = 2 (batch) x 4 (head groups of 4 heads).
Each core computes q/k/v for its 4 heads in transposed [head_dim, seq]
layout, the per-position softmax-diagonal weights, and a partial output
projection (its heads' rows of Wo), written as 2 head-pair partials that
the host sums.

Schedule (v2): the exp(scores) stream on the ACT engine is the second-
largest engine load (~180us) and is started as early as possible (~32us,
right after head 0's K/Q projections + rope). All remaining PE work
(K/Q proj heads 1-3, V proj, output proj of pair 0) is emitted as
fine-grained filler between score matmuls so the PE queue never stalls
(in-order queues: a waiting instruction blocks everything behind it) and
the PE stays at the 2.4GHz p-state. Scores PSUM is a rotating 3-buffer
of [128,1024] halves (6 banks) + a 2-buffer [128,512] pool (2 banks) for
everything else, so score matmuls never wait on exp drain.

On-chip dtype is fp16 (same PE throughput as bf16, 8x lower rounding
error - matters because exp() amplifies absolute score error), with fp32
PSUM accumulation everywhere.
"""

import numpy as np
from contextlib import ExitStack

# Problem constants (hardcoded per harness contract).
B, S, D, H, DH = 2, 2048, 2048, 16, 128
HPC = 4            # heads per core
NHL = HPC * DH     # 512 local head dims per core
KB = D // 128      # 16 contraction blocks
SB = S // 128      # 16 seq blocks of 128
SC = S // 512      # 4 seq/emb chunks of 512
NCORES = 8

_CACHE = {}


def _build_nc():
    import concourse.bass as bass
    import concourse.tile as tile
    from concourse import bacc, mybir
    from concourse.masks import make_identity

    F16 = mybir.dt.float16
    F32 = mybir.dt.float32
    AF = mybir.ActivationFunctionType

    # Bacc (not raw Bass): its compile() splits multi-sem waits into
    # event-semaphore instructions - HW allows at most 1 wait per inst.
    nc = bacc.Bacc("TRN2", target_bir_lowering=False, debug=False)

    xT = nc.dram_tensor("xT", [D, S], F16, kind="ExternalInput").ap()
    wq = nc.dram_tensor("wq", [D, NHL], F16, kind="ExternalInput").ap()
    wk = nc.dram_tensor("wk", [D, NHL], F16, kind="ExternalInput").ap()
    wv = nc.dram_tensor("wv", [D, NHL], F16, kind="ExternalInput").ap()
    wo = nc.dram_tensor("wo", [NHL, D], F16, kind="ExternalInput").ap()
    ropeA = nc.dram_tensor("ropeA", [128, S], F16, kind="ExternalInput").ap()
    ropeB = nc.dram_tensor("ropeB", [128, S], F16, kind="ExternalInput").ap()
    y = nc.dram_tensor("y", [2, S, D], F16, kind="ExternalOutput").ap()

    # partition-major views for single-shot access-pattern DMAs
    xT_p = xT.rearrange("(a p) s -> p a s", p=128)
    wq_p = wq.rearrange("(a p) m -> p a m", p=128)
    wk_p = wk.rearrange("(a p) m -> p a m", p=128)
    wv_p = wv.rearrange("(a p) m -> p a m", p=128)
    wo_p = wo.rearrange("(h p) n -> p h n", p=128)

    with tile.TileContext(nc) as tc, ExitStack() as ctx:
        pool = ctx.enter_context(tc.tile_pool(name="sb", bufs=1))
        pp = ctx.enter_context(tc.tile_pool(name="ps", bufs=1, space="PSUM"))

        # ---- constants (gpsimd SWDGE: small, keeps HWDGE queues for x/w) --
        ra = pool.tile([128, S], F16, name="ra")
        rb = pool.tile([128, S], F16, name="rb")
        nc.gpsimd.dma_start(ra[:, :], ropeA[:, :])
        nc.gpsimd.dma_start(rb[:, :], ropeB[:, :])
        ident = pool.tile([128, 128], F32, name="ident")
        make_identity(nc, ident[:, :])
        ones1 = pool.tile([128, 128], F16, name="ones1")
        nc.gpsimd.memset(ones1[:, :], 1.0)

        # ---- big input DMAs ----
        # x: 8 single-shot DMAs (one per sc half), split across the SP and
        # DVE issue queues so transfers overlap; weights one-shot each.
        xsb = pool.tile([128, KB, S], F16, name="xsb")
        wkt = pool.tile([128, KB, 512], F16, name="wt", tag="w", bufs=2)
        nc.sync.dma_start(wkt[:, :, :], wk_p[:, :, :])
        for sc in range(SC):
            sl = slice(sc * 512, (sc + 1) * 512)
            nc.sync.dma_start(xsb[:, 0:8, sl], xT_p[:, 0:8, sl])
            nc.scalar.dma_start(xsb[:, 8:16, sl], xT_p[:, 8:16, sl])
        wqt = pool.tile([128, KB, 512], F16, name="wt", tag="w", bufs=2)
        nc.sync.dma_start(wqt[:, :, :], wq_p[:, :, :])

        # ---- persistent q/k/v head tiles ([head_dim, seq] layout) ----
        qh = [pool.tile([128, S], F16, name=f"qh{h}") for h in range(HPC)]
        kh = [pool.tile([128, S], F16, name=f"kh{h}") for h in range(HPC)]
        vh = [pool.tile([128, S], F16, name=f"vh{h}") for h in range(HPC)]

        # per-head row vectors live at partition 32*h (engine ops only
        # support start partitions that are multiples of 32)
        ds_diag = pool.tile([128, S], F32, name="ds_diag")
        ds_sum = pool.tile([128, S], F16, name="ds_sum")
        w4 = pool.tile([128, S], F16, name="w4")
        expd = pool.tile([128, S], F16, name="expd")
        # 2 accum columns per sq block (exp is done in 1024-wide halves)
        sumf = [pool.tile([128, 2 * SB], F32, name=f"sumf{h}")
                for h in range(HPC)]

        def proj_chunk(wt, dests, mt, sc):
            # dests[mt][:, sc-chunk] <- (wt[:, :, mt] block).T @ x chunk
            # psum->sbuf copy on DVE (ACT is reserved for the exp stream).
            ps = pp.tile([128, 512], F32, name="mmps", tag="mm", bufs=2)
            for kb in range(KB):
                nc.tensor.matmul(
                    ps[:, :],
                    wt[:, kb, mt * 128:(mt + 1) * 128],
                    xsb[:, kb, sc * 512:(sc + 1) * 512],
                    start=(kb == 0), stop=(kb == KB - 1))
            nc.vector.tensor_copy(
                dests[mt][:, sc * 512:(sc + 1) * 512], ps[:, :])

        def rope_half(dst, c):
            # dst half (in place): top = te*cos - to*sin ; bot = te*sin+to*cos
            # ra = [cosT; cosT], rb = [-sinT; sinT]; swap = halves exchanged.
            sl = slice(c * 1024, (c + 1) * 1024)
            # SWDGE (gpsimd) keeps this 1 queue -> 1 sem; a wide HWDGE
            # sbuf->sbuf DMA fans out over many queues and blows the
            # consumer's sync-wait slot budget.
            swp = pool.tile([128, 1024], F16, name="swp", tag="swp", bufs=2)
            nc.gpsimd.dma_start(swp[0:64, :], dst[64:128, sl])
            nc.gpsimd.dma_start(swp[64:128, :], dst[0:64, sl])
            u = pool.tile([128, 1024], F16, name="u", tag="sc", bufs=2)
            nc.vector.tensor_mul(u[:, :], dst[:, sl], ra[:, sl])
            v2 = pool.tile([128, 1024], F16, name="v2", tag="sc", bufs=2)
            nc.vector.tensor_mul(v2[:, :], swp[:, :], rb[:, sl])
            nc.vector.tensor_add(dst[:, sl], u[:, :], v2[:, :])

        def diag_half(h, c):
            # ds_diag[32h, s-half] = sum_m qh[h][m,s] * kh[h][m,s]
            hp = 32 * h
            sl = slice(c * 1024, (c + 1) * 1024)
            pr = pool.tile([128, 1024], F16, name="pr", tag="pr", bufs=2)
            nc.vector.tensor_mul(pr[:, :], qh[h][:, sl], kh[h][:, sl])
            for cc in range(2):
                dps = pp.tile([128, 512], F32, name="mmps", tag="mm", bufs=2)
                nc.tensor.matmul(dps[:, :], ones1[:, :],
                                 pr[:, cc * 512:(cc + 1) * 512],
                                 start=True, stop=True)
                o = (2 * c + cc) * 512
                nc.vector.tensor_copy(ds_diag[hp:hp + 1, o:o + 512],
                                      dps[hp:hp + 1, :])

        # ====== scores stream ======
        ex = pool.tile([128, 1024], F16, name="ex")

        def scores_half(h, sq, half):
            # 2 score MMs into a rotating [128,1024] psum half, one exp
            # with fused row-sum into sumf[h][:, half*SB+sq].
            sps = pp.tile([128, 1024], F32, name="sps", tag="sco", bufs=3)
            for cc in range(2):
                ck = 2 * half + cc
                nc.tensor.matmul(sps[:, cc * 512:(cc + 1) * 512],
                                 qh[h][:, sq * 128:(sq + 1) * 128],
                                 kh[h][:, ck * 512:(ck + 1) * 512],
                                 start=True, stop=True)
            col = half * SB + sq
            nc.scalar.activation(ex[:, :], sps[:, :], AF.Exp,
                                 accum_out=sumf[h][:, col:col + 1])

        def head_sum_tail(h):
            # sum the 2 half-accums -> recip -> transpose -> [1,S] ds_sum row
            hp = 32 * h
            stot = pool.tile([128, SB], F32, name="stot", tag="rs", bufs=2)
            nc.vector.tensor_add(stot[:, :], sumf[h][:, 0:SB],
                                 sumf[h][:, SB:2 * SB])
            rs = pool.tile([128, SB], F32, name="rs", tag="rs", bufs=2)
            nc.vector.reciprocal(rs[:, :], stot[:, :])
            tps = pp.tile([16, 128], F32, name="mmps", tag="mm", bufs=2)
            nc.tensor.transpose(tps[:, :], rs[:, :], ident[:, :])
            st = pool.tile([16, 128], F16, name="st", tag="st", bufs=2)
            nc.vector.tensor_copy(st[:, :], tps[:, :])
            nc.gpsimd.dma_start(ds_sum[hp:hp + 1, :], st[:, :])

        def pair_head(p):
            # w = exp(diag) * recip(sumexp); attn = w (bcast) * v, into kh
            for h in (2 * p, 2 * p + 1):
                hp = 32 * h
                nc.scalar.activation(expd[hp:hp + 1, :],
                                     ds_diag[hp:hp + 1, :], AF.Exp)
                nc.vector.tensor_mul(w4[hp:hp + 1, :], expd[hp:hp + 1, :],
                                     ds_sum[hp:hp + 1, :])
                for ck in range(SC):
                    # K=1 outer-product broadcast of the w row to 128 parts
                    bps = pp.tile([128, 512], F32, name="mmps", tag="mm",
                                  bufs=2)
                    nc.tensor.matmul(bps[:, :], ones1[hp:hp + 1, :],
                                     w4[hp:hp + 1, ck * 512:(ck + 1) * 512],
                                     start=True, stop=True,
                                     tile_position=(hp, 0))
                    # attn scaling straight from psum (no bounce buffer)
                    nc.vector.tensor_mul(kh[h][:, ck * 512:(ck + 1) * 512],
                                         bps[:, :],
                                         vh[h][:, ck * 512:(ck + 1) * 512])

        def oproj_unit(p, sb, ncx, yts, copy_eng):
            # one 128-row x 512-col chunk of the pair-p output projection
            h0, h1 = 2 * p, 2 * p + 1
            ps = pp.tile([128, 512], F32, name="mmps", tag="mm", bufs=2)
            for i, h in enumerate((h0, h1)):
                nc.tensor.matmul(
                    ps[:, :], kh[h][:, sb * 128:(sb + 1) * 128],
                    wot[:, h, ncx * 512:(ncx + 1) * 512],
                    start=(i == 0), stop=(i == 1))
            dst = yts[:, ncx * 512:(ncx + 1) * 512]
            if copy_eng == "act":
                nc.scalar.activation(dst, ps[:, :], AF.Copy)
            else:
                nc.vector.tensor_copy(dst, ps[:, :])
            if ncx == SC - 1:
                nc.sync.dma_start(y[p, sb * 128:(sb + 1) * 128, :],
                                  yts[:, :])

        # ================= emission =================
        # Phase 0: head-0 K and Q projections, sc-interleaved so the PE can
        # start as soon as the first x chunk lands; rope halves as soon as
        # their two chunks are in.
        proj_chunk(wkt, kh, 0, 0)
        proj_chunk(wkt, kh, 0, 1)
        rope_half(kh[0], 0)
        proj_chunk(wkt, kh, 0, 2)
        proj_chunk(wkt, kh, 0, 3)
        rope_half(kh[0], 1)
        proj_chunk(wqt, qh, 0, 0)
        proj_chunk(wqt, qh, 0, 1)
        rope_half(qh[0], 0)
        diag_half(0, 0)
        proj_chunk(wqt, qh, 0, 2)
        proj_chunk(wqt, qh, 0, 3)
        rope_half(qh[0], 1)
        diag_half(0, 1)

        # Filler units: one callable each, emitted between score matmul
        # groups. Order respects cross-engine in-order queues (a DVE/ACT
        # instruction emitted before its producer would deadlock the queue).
        def proj_unit(wt_f, dests, mt, sc, ropes=(), diags=()):
            def f():
                proj_chunk(wt_f(), dests, mt, sc)
                for dst, c in ropes:
                    rope_half(dst, c)
                for h, c in diags:
                    diag_half(h, c)
            return f

        wvt = None
        wot = None

        def load_wv():
            nonlocal wvt
            wvt = pool.tile([128, KB, 512], F16, name="wt", tag="w", bufs=2)
            nc.sync.dma_start(wvt[:, :, :], wv_p[:, :, :])

        def load_wo():
            nonlocal wot
            wot = pool.tile([128, HPC, S], F16, name="wt", tag="w", bufs=2)
            nc.sync.dma_start(wot[:, :, :], wo_p[:, :, :])

        def head_units(mt):
            # 8 units: K chunks (rope halves attached after chunks 1/3),
            # then Q chunks (+diag once both q and k halves are roped).
            units = []
            for sc in range(SC):
                ropes = [(kh[mt], 0)] if sc == 1 else \
                        [(kh[mt], 1)] if sc == 3 else ()
                units.append(proj_unit(lambda: wkt, kh, mt, sc, ropes=ropes))
            for sc in range(SC):
                ropes = [(qh[mt], 0)] if sc == 1 else \
                        [(qh[mt], 1)] if sc == 3 else ()
                diags = [(mt, 0)] if sc == 1 else \
                        [(mt, 1)] if sc == 3 else ()
                units.append(proj_unit(lambda: wqt, qh, mt, sc, ropes=ropes,
                                       diags=diags))
            return units

        def v_units(mt):
            return [proj_unit(lambda: wvt, vh, mt, sc) for sc in range(SC)]

        # h0 stream fillers: K/Q projections of heads 1 and 2 (16 units)
        fill_h0 = head_units(1) + head_units(2)
        # h1: head 3 K, wv load (overlaps Q3), Q3, then V heads 0,1
        k3q3 = head_units(3)
        fill_h1 = k3q3[:4] + [load_wv] + k3q3[4:] + v_units(0) + v_units(1)
        # h2: wo load first (oproj units start at sq2), then V heads 2,3
        # (their copies are needed only by pair_head(1))
        fill_h2 = [load_wo] + v_units(2) + v_units(3)

        for h, fill in ((0, fill_h0), (1, fill_h1)):
            for sq in range(SB):
                if fill:
                    fill.pop(0)()
                scores_half(h, sq, 0)
                if len(fill) > SB - 1 - sq:
                    fill.pop(0)()
                scores_half(h, sq, 1)
            for f in fill:
                f()
            head_sum_tail(h)
        pair_head(0)

        # scores h2/h3: remaining V-proj fillers, then pair-0 oproj units
        yts = {}
        ofill = [(0, sb, ncx) for sb in range(SB) for ncx in range(SC)]

        def oproj_pop(n, copy_eng="dve"):
            for _ in range(n):
                if not ofill:
                    return
                p, sb, ncx = ofill.pop(0)
                if ncx == 0:
                    yts[p] = pool.tile([128, S], F16, name="yt",
                                       tag="yt", bufs=2)
                oproj_unit(p, sb, ncx, yts[p], copy_eng)

        for h, fill in ((2, fill_h2), (3, [])):
            for sq in range(SB):
                if fill:
                    fill.pop(0)()
                    oproj_pop(1 if sq >= 2 else 0)
                else:
                    oproj_pop(3)
                scores_half(h, sq, 0)
                scores_half(h, sq, 1)
            head_sum_tail(h)
        pair_head(1)

        # tail: leftover pair-0 units, then pair-1 output projection.
        # psum->sbuf copies alternate DVE/ACT (ACT is idle by now).
        ofill += [(1, sb, ncx) for sb in range(SB) for ncx in range(SC)]
        i = 0
        while ofill:
            oproj_pop(1, "act" if i % 2 else "dve")
            i += 1

    nc.compile()
    return nc


def _get_nc():
    if "nc" not in _CACHE:
        _CACHE["nc"] = _build_nc()
    return _CACHE["nc"]


_PERM = np.concatenate([np.arange(0, DH, 2), np.arange(1, DH, 2)])


def _host_inputs(x, rope_cos, rope_sin, Wq, Wk, Wv, Wo):
    """Build the 8 per-core input maps."""
    f16 = np.float16
    cosT = np.ascontiguousarray(np.asarray(rope_cos, np.float32)[0, :, 0, :].T)
    sinT = np.ascontiguousarray(np.asarray(rope_sin, np.float32)[0, :, 0, :].T)
    ra = np.concatenate([cosT, cosT], 0).astype(f16)
    rb = np.concatenate([-sinT, sinT], 0).astype(f16)

    Wq = np.asarray(Wq, np.float32)
    Wk = np.asarray(Wk, np.float32)
    Wv = np.asarray(Wv, np.float32)
    Wo = np.asarray(Wo, np.float32)
    x = np.asarray(x, np.float32)

    xTb = [np.ascontiguousarray(x[b].T).astype(f16) for b in range(B)]
    scale = DH ** -0.5

    in_maps = []
    for core in range(NCORES):
        b, g = divmod(core, HPC)
        hs = g * HPC
        rows = np.concatenate(
            [h * DH + _PERM for h in range(hs, hs + HPC)])      # deinterleave
        rows_v = np.arange(hs * DH, (hs + HPC) * DH)
        in_maps.append({
            "xT": xTb[b],
            "wq": np.ascontiguousarray((Wq[rows] * scale).T).astype(f16),
            "wk": np.ascontiguousarray(Wk[rows].T).astype(f16),
            "wv": np.ascontiguousarray(Wv[rows_v].T).astype(f16),
            "wo": np.ascontiguousarray(Wo[:, rows_v].T).astype(f16),
            "ropeA": ra,
            "ropeB": rb,
        })
    return in_maps


def kernel(x, rope_cos, rope_sin, Wq, Wk, Wv, Wo, _trace=False, _trace_cores=None):
    from concourse.bass_utils import run_bass_kernel_spmd

    nc = _get_nc()
    in_maps = _host_inputs(x, rope_cos, rope_sin, Wq, Wk, Wv, Wo)
    res = run_bass_kernel_spmd(nc, in_maps, list(range(NCORES)),
                               trace=_trace, trace_cores=_trace_cores)
    _CACHE["last_result"] = res

    out = np.zeros((B, S, D), np.float32)
    for core in range(NCORES):
        b = core // HPC
        out[b] += res.results[core]["y"].astype(np.float32).sum(axis=0)
    return out


# revision 19
# speedup vs baseline: 1.2070x; 1.0062x over previous
"""Trainium2 Bass kernel for the MHA-with-diagonal-softmax module.

Computation (per batch b):
    q = rope(x @ Wq.T), k = rope(x @ Wk.T), v = x @ Wv.T      (per head, DH=128)
    sumexp[s,h] = sum_k exp(q_h[s] . k_h[k] * DH^-0.5)
    diag[s,h]   = q_h[s] . k_h[s] * DH^-0.5
    w = exp(diag) / sumexp
    out = (w * v) @ Wo.T

Sharding: 8 cores = 2 (batch) x 4 (head groups of 4 heads).
Each core computes q/k/v for its 4 heads in transposed [head_dim, seq]
layout, the per-position softmax-diagonal weights, and a partial output
projection (its heads' rows of Wo), written as 2 head-pair partials that
the host sums.

Schedule (v2): the exp(scores) stream on the ACT engine is the second-
largest engine load (~180us) and is started as early as possible (~32us,
right after head 0's K/Q projections + rope). All remaining PE work
(K/Q proj heads 1-3, V proj, output proj of pair 0) is emitted as
fine-grained filler between score matmuls so the PE queue never stalls
(in-order queues: a waiting instruction blocks everything behind it) and
the PE stays at the 2.4GHz p-state. Scores PSUM is a rotating 3-buffer
of [128,1024] halves (6 banks) + a 2-buffer [128,512] pool (2 banks) for
everything else, so score matmuls never wait on exp drain.

On-chip dtype is fp16 (same PE throughput as bf16, 8x lower rounding
error - matters because exp() amplifies absolute score error), with fp32
PSUM accumulation everywhere.
"""

import numpy as np
from contextlib import ExitStack

# Problem constants (hardcoded per harness contract).
B, S, D, H, DH = 2, 2048, 2048, 16, 128
HPC = 4            # heads per core
NHL = HPC * DH     # 512 local head dims per core
KB = D // 128      # 16 contraction blocks
SB = S // 128      # 16 seq blocks of 128
SC = S // 512      # 4 seq/emb chunks of 512
NCORES = 8

_CACHE = {}


def _build_nc():
    import concourse.bass as bass
    import concourse.tile as tile
    from concourse import bacc, mybir
    from concourse.masks import make_identity

    F16 = mybir.dt.float16
    F32 = mybir.dt.float32
    AF = mybir.ActivationFunctionType

    # Bacc (not raw Bass): its compile() splits multi-sem waits into
    # event-semaphore instructions - HW allows at most 1 wait per inst.
    nc = bacc.Bacc("TRN2", target_bir_lowering=False, debug=False)

    xT = nc.dram_tensor("xT", [D, S], F16, kind="ExternalInput").ap()
    wq = nc.dram_tensor("wq", [D, NHL], F16, kind="ExternalInput").ap()
    wk = nc.dram_tensor("wk", [D, NHL], F16, kind="ExternalInput").ap()
    wv = nc.dram_tensor("wv", [D, NHL], F16, kind="ExternalInput").ap()
    wo = nc.dram_tensor("wo", [NHL, D], F16, kind="ExternalInput").ap()
    ropeA = nc.dram_tensor("ropeA", [128, S], F16, kind="ExternalInput").ap()
    ropeB = nc.dram_tensor("ropeB", [128, S], F16, kind="ExternalInput").ap()
    y = nc.dram_tensor("y", [2, S, D], F16, kind="ExternalOutput").ap()

    # partition-major views for single-shot access-pattern DMAs
    xT_p = xT.rearrange("(a p) s -> p a s", p=128)
    wq_p = wq.rearrange("(a p) m -> p a m", p=128)
    wk_p = wk.rearrange("(a p) m -> p a m", p=128)
    wv_p = wv.rearrange("(a p) m -> p a m", p=128)
    wo_p = wo.rearrange("(h p) n -> p h n", p=128)

    with tile.TileContext(nc) as tc, ExitStack() as ctx:
        pool = ctx.enter_context(tc.tile_pool(name="sb", bufs=1))
        pp = ctx.enter_context(tc.tile_pool(name="ps", bufs=1, space="PSUM"))

        # ---- constants (gpsimd SWDGE: small, keeps HWDGE queues for x/w) --
        ra = pool.tile([128, S], F16, name="ra")
        rb = pool.tile([128, S], F16, name="rb")
        nc.gpsimd.dma_start(ra[:, :], ropeA[:, :])
        nc.gpsimd.dma_start(rb[:, :], ropeB[:, :])
        ident = pool.tile([128, 128], F32, name="ident")
        make_identity(nc, ident[:, :])
        ones1 = pool.tile([128, 128], F16, name="ones1")
        nc.gpsimd.memset(ones1[:, :], 1.0)

        # ---- big input DMAs ----
        # wk/wq first (small, unblock the PE), then x one block per kb so
        # the head-0 K/Q projections can accumulate into held PSUM tiles as
        # each block lands. Issues alternate SP/ACT queues.
        xsb = pool.tile([128, KB, S], F16, name="xsb")
        wkt = pool.tile([128, KB, 512], F16, name="wt", tag="w", bufs=2)
        wqt = pool.tile([128, KB, 512], F16, name="wt", tag="w", bufs=2)
        nc.sync.dma_start(wkt[:, :, :], wk_p[:, :, :])
        nc.scalar.dma_start(wqt[:, :, :], wq_p[:, :, :])
        for kb in range(KB):
            eng = nc.sync if kb % 2 == 0 else nc.scalar
            eng.dma_start(xsb[:, kb, :], xT_p[:, kb, :])

        # ---- persistent q/k/v head tiles ([head_dim, seq] layout) ----
        qh = [pool.tile([128, S], F16, name=f"qh{h}") for h in range(HPC)]
        kh = [pool.tile([128, S], F16, name=f"kh{h}") for h in range(HPC)]
        vh = [pool.tile([128, S], F16, name=f"vh{h}") for h in range(HPC)]

        # per-head row vectors live at partition 32*h (engine ops only
        # support start partitions that are multiples of 32)
        ds_diag = pool.tile([128, S], F32, name="ds_diag")
        ds_sum = pool.tile([128, S], F16, name="ds_sum")
        w4 = pool.tile([128, S], F16, name="w4")
        expd = pool.tile([128, S], F16, name="expd")
        # 2 accum columns per sq block (exp is done in 1024-wide halves)
        sumf = [pool.tile([128, 2 * SB], F32, name=f"sumf{h}")
                for h in range(HPC)]

        def proj_chunk(wt, dests, mt, sc):
            # dests[mt][:, sc-chunk] <- (wt[:, :, mt] block).T @ x chunk
            # psum->sbuf copy on DVE (ACT is reserved for the exp stream).
            ps = pp.tile([128, 512], F32, name="mmps", tag="mm", bufs=4)
            for kb in range(KB):
                nc.tensor.matmul(
                    ps[:, :],
                    wt[:, kb, mt * 128:(mt + 1) * 128],
                    xsb[:, kb, sc * 512:(sc + 1) * 512],
                    start=(kb == 0), stop=(kb == KB - 1))
            nc.vector.tensor_copy(
                dests[mt][:, sc * 512:(sc + 1) * 512], ps[:, :])

        def rope_half(dst, c):
            # dst half (in place): top = te*cos - to*sin ; bot = te*sin+to*cos
            # ra = [cosT; cosT], rb = [-sinT; sinT]; swap = halves exchanged.
            sl = slice(c * 1024, (c + 1) * 1024)
            # SWDGE (gpsimd) keeps this 1 queue -> 1 sem; a wide HWDGE
            # sbuf->sbuf DMA fans out over many queues and blows the
            # consumer's sync-wait slot budget.
            swp = pool.tile([128, 1024], F16, name="swp", tag="swp", bufs=2)
            nc.gpsimd.dma_start(swp[0:64, :], dst[64:128, sl])
            nc.gpsimd.dma_start(swp[64:128, :], dst[0:64, sl])
            u = pool.tile([128, 1024], F16, name="u", tag="sc", bufs=2)
            nc.vector.tensor_mul(u[:, :], dst[:, sl], ra[:, sl])
            v2 = pool.tile([128, 1024], F16, name="v2", tag="sc", bufs=2)
            nc.vector.tensor_mul(v2[:, :], swp[:, :], rb[:, sl])
            nc.vector.tensor_add(dst[:, sl], u[:, :], v2[:, :])

        def diag_half(h, c):
            # ds_diag[32h, s-half] = sum_m qh[h][m,s] * kh[h][m,s]
            hp = 32 * h
            sl = slice(c * 1024, (c + 1) * 1024)
            pr = pool.tile([128, 1024], F16, name="pr", tag="pr", bufs=2)
            nc.vector.tensor_mul(pr[:, :], qh[h][:, sl], kh[h][:, sl])
            for cc in range(2):
                dps = pp.tile([128, 512], F32, name="mmps", tag="mm", bufs=4)
                nc.tensor.matmul(dps[:, :], ones1[:, :],
                                 pr[:, cc * 512:(cc + 1) * 512],
                                 start=True, stop=True)
                o = (2 * c + cc) * 512
                nc.vector.tensor_copy(ds_diag[hp:hp + 1, o:o + 512],
                                      dps[hp:hp + 1, :])

        # ====== scores stream ======
        ex = pool.tile([128, 1024], F16, name="ex")

        def scores_half(h, sq, half):
            # 2 score MMs into a rotating [128,1024] psum half, one exp
            # with fused row-sum into sumf[h][:, half*SB+sq].
            sps = pp.tile([128, 1024], F32, name="sps", tag="sco", bufs=2)
            for cc in range(2):
                ck = 2 * half + cc
                nc.tensor.matmul(sps[:, cc * 512:(cc + 1) * 512],
                                 qh[h][:, sq * 128:(sq + 1) * 128],
                                 kh[h][:, ck * 512:(ck + 1) * 512],
                                 start=True, stop=True)
            col = half * SB + sq
            nc.scalar.activation(ex[:, :], sps[:, :], AF.Exp,
                                 accum_out=sumf[h][:, col:col + 1])

        rsq = {}

        def head_sum_pre(h):
            # DVE-only piece: sum the 2 half-accums, reciprocal
            stot = pool.tile([128, SB], F32, name="stot", tag="rs", bufs=2)
            nc.vector.tensor_add(stot[:, :], sumf[h][:, 0:SB],
                                 sumf[h][:, SB:2 * SB])
            rsq[h] = pool.tile([128, SB], F32, name="rs", tag="rs", bufs=2)
            nc.vector.reciprocal(rsq[h][:, :], stot[:, :])

        def head_sum_post(h):
            # transpose -> [1,S] ds_sum row (PE piece, emitted after a
            # filler so the PE queue has work while the DVE piece resolves)
            hp = 32 * h
            tps = pp.tile([16, 128], F32, name="mmps", tag="mm", bufs=4)
            nc.tensor.transpose(tps[:, :], rsq[h][:, :], ident[:, :])
            st = pool.tile([16, 128], F16, name="st", tag="st", bufs=2)
            nc.vector.tensor_copy(st[:, :], tps[:, :])
            nc.gpsimd.dma_start(ds_sum[hp:hp + 1, :], st[:, :])

        def pair_head(p, units):
            # w = exp(diag) * recip(sumexp); attn = w (bcast) * v, into kh.
            # ACT exps first (no PE coupling), then broadcast+scale chunk
            # groups interleaved with independent PE units.
            for h in (2 * p, 2 * p + 1):
                hp = 32 * h
                nc.scalar.activation(expd[hp:hp + 1, :],
                                     ds_diag[hp:hp + 1, :], AF.Exp)
            if units:
                units.pop(0)()
            for h in (2 * p, 2 * p + 1):
                hp = 32 * h
                nc.vector.tensor_mul(w4[hp:hp + 1, :], expd[hp:hp + 1, :],
                                     ds_sum[hp:hp + 1, :])
            for ck in range(SC):
                for h in (2 * p, 2 * p + 1):
                    hp = 32 * h
                    # K=1 outer-product broadcast of the w row to 128 parts
                    bps = pp.tile([128, 512], F32, name="mmps", tag="mm",
                                  bufs=4)
                    nc.tensor.matmul(bps[:, :], ones1[hp:hp + 1, :],
                                     w4[hp:hp + 1, ck * 512:(ck + 1) * 512],
                                     start=True, stop=True,
                                     tile_position=(hp, 0))
                    # attn scaling straight from psum (no bounce buffer)
                    nc.vector.tensor_mul(kh[h][:, ck * 512:(ck + 1) * 512],
                                         bps[:, :],
                                         vh[h][:, ck * 512:(ck + 1) * 512])
                if units:
                    units.pop(0)()

        def oproj_unit(p, sb, ncx, yts, copy_eng):
            # one 128-row x 512-col chunk of the pair-p output projection
            h0, h1 = 2 * p, 2 * p + 1
            ps = pp.tile([128, 512], F32, name="mmps", tag="mm", bufs=4)
            for i, h in enumerate((h0, h1)):
                nc.tensor.matmul(
                    ps[:, :], kh[h][:, sb * 128:(sb + 1) * 128],
                    wot[:, h, ncx * 512:(ncx + 1) * 512],
                    start=(i == 0), stop=(i == 1))
            dst = yts[:, ncx * 512:(ncx + 1) * 512]
            if copy_eng == "act":
                nc.scalar.activation(dst, ps[:, :], AF.Copy)
            else:
                nc.vector.tensor_copy(dst, ps[:, :])
            if ncx == SC - 1:
                nc.sync.dma_start(y[p, sb * 128:(sb + 1) * 128, :],
                                  yts[:, :])

        # ================= emission =================
        # Phase 0: head-0 K and Q projections in kb-major order - one
        # accumulation step into 6 held PSUM tiles per x block as it lands,
        # so the PE tracks the x DMA stream instead of waiting for all of x.
        kA = pp.tile([128, 1024], F32, name="sps", tag="sco", bufs=2)
        kB = pp.tile([128, 1024], F32, name="sps", tag="sco", bufs=2)
        q4 = [pp.tile([128, 512], F32, name="mmps", tag="mm", bufs=4)
              for _ in range(4)]
        for kb in range(KB):
            st_, sp_ = (kb == 0), (kb == KB - 1)
            for sc in range(2):
                nc.tensor.matmul(kA[:, sc * 512:(sc + 1) * 512],
                                 wkt[:, kb, 0:128],
                                 xsb[:, kb, sc * 512:(sc + 1) * 512],
                                 start=st_, stop=sp_)
            for sc in range(2, 4):
                nc.tensor.matmul(kB[:, (sc - 2) * 512:(sc - 1) * 512],
                                 wkt[:, kb, 0:128],
                                 xsb[:, kb, sc * 512:(sc + 1) * 512],
                                 start=st_, stop=sp_)
            for sc in range(4):
                nc.tensor.matmul(q4[sc][:, :], wqt[:, kb, 0:128],
                                 xsb[:, kb, sc * 512:(sc + 1) * 512],
                                 start=st_, stop=sp_)
        # drain + rope, ordered to unblock the first score matmuls earliest
        nc.vector.tensor_copy(kh[0][:, 0:1024], kA[:, :])
        rope_half(kh[0], 0)
        nc.vector.tensor_copy(kh[0][:, 1024:2048], kB[:, :])
        rope_half(kh[0], 1)
        nc.vector.tensor_copy(qh[0][:, 0:512], q4[0][:, :])
        nc.vector.tensor_copy(qh[0][:, 512:1024], q4[1][:, :])
        rope_half(qh[0], 0)
        nc.vector.tensor_copy(qh[0][:, 1024:1536], q4[2][:, :])
        nc.vector.tensor_copy(qh[0][:, 1536:2048], q4[3][:, :])
        rope_half(qh[0], 1)

        # Filler units: one callable each, emitted between score matmul
        # groups. Order respects cross-engine in-order queues (a DVE/ACT
        # instruction emitted before its producer would deadlock the queue).
        def proj_unit(wt_f, dests, mt, sc, ropes=(), diags=()):
            def f():
                proj_chunk(wt_f(), dests, mt, sc)
                for dst, c in ropes:
                    rope_half(dst, c)
                for h, c in diags:
                    diag_half(h, c)
            return f

        wvt = None
        wot = None

        def load_wv():
            nonlocal wvt
            wvt = pool.tile([128, KB, 512], F16, name="wt", tag="w", bufs=2)
            nc.sync.dma_start(wvt[:, :, :], wv_p[:, :, :])

        def load_wo():
            nonlocal wot
            wot = pool.tile([128, HPC, S], F16, name="wt", tag="w", bufs=2)
            nc.sync.dma_start(wot[:, :, :], wo_p[:, :, :])

        def head_units(mt):
            # 8 units: K chunks (rope halves attached after chunks 1/3),
            # then Q chunks (+diag once both q and k halves are roped).
            units = []
            for sc in range(SC):
                ropes = [(kh[mt], 0)] if sc == 1 else \
                        [(kh[mt], 1)] if sc == 3 else ()
                units.append(proj_unit(lambda: wkt, kh, mt, sc, ropes=ropes))
            for sc in range(SC):
                ropes = [(qh[mt], 0)] if sc == 1 else \
                        [(qh[mt], 1)] if sc == 3 else ()
                diags = [(mt, 0)] if sc == 1 else \
                        [(mt, 1)] if sc == 3 else ()
                units.append(proj_unit(lambda: wqt, qh, mt, sc, ropes=ropes,
                                       diags=diags))
            return units

        def v_units(mt):
            return [proj_unit(lambda: wvt, vh, mt, sc) for sc in range(SC)]

        def diag_unit(h):
            def f():
                diag_half(h, 0)
                diag_half(h, 1)
            return f

        # h0 stream fillers: deferred head-0 diag, K/Q of heads 1 and 2
        fill_h0 = [diag_unit(0)] + head_units(1) + head_units(2)
        # h1: head 3 K, wv load (overlaps Q3), Q3, wo load, then V heads 0,1
        k3q3 = head_units(3)
        fill_h1 = (k3q3[:4] + [load_wv] + k3q3[4:] + [load_wo]
                   + v_units(0) + v_units(1))
        # h2: V heads 2,3 (their copies are needed only by pair_head(1))
        fill_h2 = v_units(2) + v_units(3)

        yts = {}
        ofill = [(0, sb, ncx) for sb in range(SB) for ncx in range(SC)]

        def oproj_pop(n, copy_eng="dve", keep=0):
            for _ in range(n):
                if len(ofill) <= keep:
                    return
                p, sb, ncx = ofill.pop(0)
                if ncx == 0:
                    yts[p] = pool.tile([128, S], F16, name="yt",
                                       tag="yt", bufs=2)
                oproj_unit(p, sb, ncx, yts[p], copy_eng)

        def stream(h, fill, per_sq_oproj=0):
            for sq in range(SB):
                if fill:
                    fill.pop(0)()
                if per_sq_oproj:
                    # hold back 5 pair-0 units to feed the PE through the
                    # pair-1 boundary chain
                    oproj_pop(per_sq_oproj, keep=5)
                scores_half(h, sq, 0)
                if len(fill) > SB - 1 - sq:
                    fill.pop(0)()
                scores_half(h, sq, 1)
            head_sum_pre(h)

        stream(0, fill_h0)
        fill_h1.pop(0)()                       # K3c0: PE work for the gap
        head_sum_post(0)
        stream(1, fill_h1)
        fill_h2.pop(0)()                       # V2c0
        head_sum_post(1)
        pair_head(0, fill_h2)                  # interleaves V2c1..V3c1
        stream(2, fill_h2, per_sq_oproj=2)     # V3c2,V3c3 + pair-0 oproj
        oproj_pop(1, keep=5)
        head_sum_post(2)
        stream(3, [], per_sq_oproj=2)

        def reserve_unit():
            def f():
                oproj_pop(1)
            return f

        head_sum_post(3)
        pair_head(1, [reserve_unit() for _ in range(5)])

        # tail: pair-1 output projection; psum->sbuf copies alternate
        # DVE/ACT (ACT is idle by now).
        ofill += [(1, sb, ncx) for sb in range(SB) for ncx in range(SC)]
        i = 0
        while ofill:
            oproj_pop(1, "act" if i % 2 else "dve")
            i += 1

    nc.compile()
    return nc


def _get_nc():
    if "nc" not in _CACHE:
        _CACHE["nc"] = _build_nc()
    return _CACHE["nc"]


_PERM = np.concatenate([np.arange(0, DH, 2), np.arange(1, DH, 2)])


def _host_inputs(x, rope_cos, rope_sin, Wq, Wk, Wv, Wo):
    """Build the 8 per-core input maps."""
    f16 = np.float16
    cosT = np.ascontiguousarray(np.asarray(rope_cos, np.float32)[0, :, 0, :].T)
    sinT = np.ascontiguousarray(np.asarray(rope_sin, np.float32)[0, :, 0, :].T)
    ra = np.concatenate([cosT, cosT], 0).astype(f16)
    rb = np.concatenate([-sinT, sinT], 0).astype(f16)

    Wq = np.asarray(Wq, np.float32)
    Wk = np.asarray(Wk, np.float32)
    Wv = np.asarray(Wv, np.float32)
    Wo = np.asarray(Wo, np.float32)
    x = np.asarray(x, np.float32)

    xTb = [np.ascontiguousarray(x[b].T).astype(f16) for b in range(B)]
    scale = DH ** -0.5

    in_maps = []
    for core in range(NCORES):
        b, g = divmod(core, HPC)
        hs = g * HPC
        rows = np.concatenate(
            [h * DH + _PERM for h in range(hs, hs + HPC)])      # deinterleave
        rows_v = np.arange(hs * DH, (hs + HPC) * DH)
        in_maps.append({
            "xT": xTb[b],
            "wq": np.ascontiguousarray((Wq[rows] * scale).T).astype(f16),
            "wk": np.ascontiguousarray(Wk[rows].T).astype(f16),
            "wv": np.ascontiguousarray(Wv[rows_v].T).astype(f16),
            "wo": np.ascontiguousarray(Wo[:, rows_v].T).astype(f16),
            "ropeA": ra,
            "ropeB": rb,
        })
    return in_maps


def kernel(x, rope_cos, rope_sin, Wq, Wk, Wv, Wo, _trace=False, _trace_cores=None):
    from concourse.bass_utils import run_bass_kernel_spmd

    nc = _get_nc()
    in_maps = _host_inputs(x, rope_cos, rope_sin, Wq, Wk, Wv, Wo)
    res = run_bass_kernel_spmd(nc, in_maps, list(range(NCORES)),
                               trace=_trace, trace_cores=_trace_cores)
    _CACHE["last_result"] = res

    out = np.zeros((B, S, D), np.float32)
    for core in range(NCORES):
        b = core // HPC
        out[b] += res.results[core]["y"].astype(np.float32).sum(axis=0)
    return out


# revision 26
# speedup vs baseline: 1.2428x; 1.0297x over previous
"""Trainium2 Bass kernel for the MHA-with-diagonal-softmax module.

Computation (per batch b):
    q = rope(x @ Wq.T), k = rope(x @ Wk.T), v = x @ Wv.T      (per head, DH=128)
    sumexp[s,h] = sum_k exp(q_h[s] . k_h[k] * DH^-0.5)
    diag[s,h]   = q_h[s] . k_h[s] * DH^-0.5
    w = exp(diag) / sumexp
    out = (w * v) @ Wo.T

Sharding: 8 cores = 2 (batch) x 4 (head groups of 4 heads).
Each core computes q/k/v for its 4 heads in transposed [head_dim, seq]
layout, the per-position softmax-diagonal weights, and a partial output
projection (its heads' rows of Wo), written as 2 head-pair partials that
the host sums.

Schedule (v2): the exp(scores) stream on the ACT engine is the second-
largest engine load (~180us) and is started as early as possible (~32us,
right after head 0's K/Q projections + rope). All remaining PE work
(K/Q proj heads 1-3, V proj, output proj of pair 0) is emitted as
fine-grained filler between score matmuls so the PE queue never stalls
(in-order queues: a waiting instruction blocks everything behind it) and
the PE stays at the 2.4GHz p-state. Scores PSUM is a rotating 3-buffer
of [128,1024] halves (6 banks) + a 2-buffer [128,512] pool (2 banks) for
everything else, so score matmuls never wait on exp drain.

On-chip dtype is fp16 (same PE throughput as bf16, 8x lower rounding
error - matters because exp() amplifies absolute score error), with fp32
PSUM accumulation everywhere.
"""

import numpy as np
from contextlib import ExitStack

# Problem constants (hardcoded per harness contract).
B, S, D, H, DH = 2, 2048, 2048, 16, 128
HPC = 4            # heads per core
NHL = HPC * DH     # 512 local head dims per core
KB = D // 128      # 16 contraction blocks
SB = S // 128      # 16 seq blocks of 128
SC = S // 512      # 4 seq/emb chunks of 512
NCORES = 8

_CACHE = {}


def _build_nc():
    import concourse.bass as bass
    import concourse.tile as tile
    from concourse import bacc, mybir
    from concourse.masks import make_identity

    F16 = mybir.dt.float16
    F32 = mybir.dt.float32
    AF = mybir.ActivationFunctionType

    # Bacc (not raw Bass): its compile() splits multi-sem waits into
    # event-semaphore instructions - HW allows at most 1 wait per inst.
    nc = bacc.Bacc("TRN2", target_bir_lowering=False, debug=False)

    xT = nc.dram_tensor("xT", [D, S], F16, kind="ExternalInput").ap()
    wq = nc.dram_tensor("wq", [D, NHL], F16, kind="ExternalInput").ap()
    wk = nc.dram_tensor("wk", [D, NHL], F16, kind="ExternalInput").ap()
    wv = nc.dram_tensor("wv", [D, NHL], F16, kind="ExternalInput").ap()
    wo = nc.dram_tensor("wo", [NHL, D], F16, kind="ExternalInput").ap()
    ropeA = nc.dram_tensor("ropeA", [128, S], F16, kind="ExternalInput").ap()
    ropeB = nc.dram_tensor("ropeB", [128, S], F16, kind="ExternalInput").ap()
    y = nc.dram_tensor("y", [2, S, D], F16, kind="ExternalOutput").ap()

    # partition-major views for single-shot access-pattern DMAs
    xT_r = xT.rearrange("(a p) s -> a p s", p=128)
    wq_p = wq.rearrange("(a p) m -> p a m", p=128)
    wk_p = wk.rearrange("(a p) m -> p a m", p=128)
    wv_p = wv.rearrange("(a p) m -> p a m", p=128)
    wo_p = wo.rearrange("(h p) n -> p h n", p=128)

    with tile.TileContext(nc) as tc, ExitStack() as ctx:
        pool = ctx.enter_context(tc.tile_pool(name="sb", bufs=1))
        pp = ctx.enter_context(tc.tile_pool(name="ps", bufs=1, space="PSUM"))

        # ---- constants (gpsimd SWDGE: small, keeps HWDGE queues for x/w) --
        ra = pool.tile([128, S], F16, name="ra")
        rb = pool.tile([128, S], F16, name="rb")
        nc.gpsimd.dma_start(ra[:, :], ropeA[:, :])
        nc.gpsimd.dma_start(rb[:, :], ropeB[:, :])
        ident = pool.tile([128, 128], F32, name="ident")
        make_identity(nc, ident[:, :])
        ones1 = pool.tile([128, 128], F16, name="ones1")
        nc.gpsimd.memset(ones1[:, :], 1.0)

        # ---- big input DMAs ----
        # wk/wq first (small, unblock the PE), then x one block per kb so
        # the head-0 K/Q projections can accumulate into held PSUM tiles as
        # each block lands. One TILE per kb block: the Tile framework tracks
        # dependencies per tile, so a single xsb tile would make the first
        # matmul wait for all 16 DMAs. Issues alternate SP/ACT queues.
        xsb = [pool.tile([128, S], F16, name=f"xsb{kb}") for kb in range(KB)]
        wkt = pool.tile([128, KB, 512], F16, name="wt", tag="w", bufs=2)
        wqt = pool.tile([128, KB, 512], F16, name="wt", tag="w", bufs=2)
        nc.sync.dma_start(wkt[:, :, :], wk_p[:, :, :])
        nc.scalar.dma_start(wqt[:, :, :], wq_p[:, :, :])
        for kb in range(KB):
            eng = nc.sync if kb % 2 == 0 else nc.scalar
            eng.dma_start(xsb[kb][:, :], xT_r[kb])

        # ---- persistent q/k/v head tiles ([head_dim, seq] layout) ----
        qh = [pool.tile([128, S], F16, name=f"qh{h}") for h in range(HPC)]
        kh = [pool.tile([128, S], F16, name=f"kh{h}") for h in range(HPC)]
        vh = [pool.tile([128, S], F16, name=f"vh{h}") for h in range(HPC)]

        # per-head row vectors live at partition 32*h (engine ops only
        # support start partitions that are multiples of 32)
        ds_diag = pool.tile([128, S], F32, name="ds_diag")
        ds_sum = pool.tile([128, S], F16, name="ds_sum")
        w4 = pool.tile([128, S], F16, name="w4")
        expd = pool.tile([128, S], F16, name="expd")
        # 2 accum columns per sq block (exp is done in 1024-wide halves)
        sumf = [pool.tile([128, 2 * SB], F32, name=f"sumf{h}")
                for h in range(HPC)]

        def proj_mms(wt, mt, sc):
            # (wt[:, :, mt] block).T @ x chunk -> a rotating psum tile
            ps = pp.tile([128, 512], F32, name="mmps", tag="mm", bufs=4)
            for kb in range(KB):
                nc.tensor.matmul(
                    ps[:, :],
                    wt[:, kb, mt * 128:(mt + 1) * 128],
                    xsb[kb][:, sc * 512:(sc + 1) * 512],
                    start=(kb == 0), stop=(kb == KB - 1))
            return ps

        def proj_chunk(wt, dests, mt, sc):
            # psum->sbuf copy on DVE (ACT is reserved for the exp stream).
            ps = proj_mms(wt, mt, sc)
            nc.vector.tensor_copy(
                dests[mt][:, sc * 512:(sc + 1) * 512], ps[:, :])

        def rope_half(dst, c):
            # dst half (in place): top = te*cos - to*sin ; bot = te*sin+to*cos
            # ra = [cosT; cosT], rb = [-sinT; sinT]; swap = halves exchanged.
            sl = slice(c * 1024, (c + 1) * 1024)
            # SWDGE (gpsimd) keeps this 1 queue -> 1 sem; a wide HWDGE
            # sbuf->sbuf DMA fans out over many queues and blows the
            # consumer's sync-wait slot budget.
            swp = pool.tile([128, 1024], F16, name="swp", tag="swp", bufs=2)
            nc.gpsimd.dma_start(swp[0:64, :], dst[64:128, sl])
            nc.gpsimd.dma_start(swp[64:128, :], dst[0:64, sl])
            u = pool.tile([128, 1024], F16, name="u", tag="sc", bufs=2)
            nc.vector.tensor_mul(u[:, :], dst[:, sl], ra[:, sl])
            v2 = pool.tile([128, 1024], F16, name="v2", tag="sc", bufs=2)
            nc.vector.tensor_mul(v2[:, :], swp[:, :], rb[:, sl])
            nc.vector.tensor_add(dst[:, sl], u[:, :], v2[:, :])

        def diag_half(h, c):
            # ds_diag[32h, s-half] = sum_m qh[h][m,s] * kh[h][m,s]
            hp = 32 * h
            sl = slice(c * 1024, (c + 1) * 1024)
            pr = pool.tile([128, 1024], F16, name="pr", tag="pr", bufs=2)
            nc.vector.tensor_mul(pr[:, :], qh[h][:, sl], kh[h][:, sl])
            for cc in range(2):
                dps = pp.tile([128, 512], F32, name="mmps", tag="mm", bufs=4)
                nc.tensor.matmul(dps[:, :], ones1[:, :],
                                 pr[:, cc * 512:(cc + 1) * 512],
                                 start=True, stop=True)
                o = (2 * c + cc) * 512
                nc.vector.tensor_copy(ds_diag[hp:hp + 1, o:o + 512],
                                      dps[hp:hp + 1, :])

        # ====== scores stream ======
        ex = pool.tile([128, 1024], F16, name="ex")

        def scores_half(h, sq, half):
            # 2 score MMs into a rotating [128,1024] psum half, one exp
            # with fused row-sum into sumf[h][:, half*SB+sq].
            sps = pp.tile([128, 1024], F32, name="sps", tag="sco", bufs=2)
            for cc in range(2):
                ck = 2 * half + cc
                nc.tensor.matmul(sps[:, cc * 512:(cc + 1) * 512],
                                 qh[h][:, sq * 128:(sq + 1) * 128],
                                 kh[h][:, ck * 512:(ck + 1) * 512],
                                 start=True, stop=True)
            col = half * SB + sq
            nc.scalar.activation(ex[:, :], sps[:, :], AF.Exp,
                                 accum_out=sumf[h][:, col:col + 1])

        rsq = {}

        def head_sum_pre(h):
            # DVE-only piece: sum the 2 half-accums, reciprocal
            stot = pool.tile([128, SB], F32, name="stot", tag="rs", bufs=2)
            nc.vector.tensor_add(stot[:, :], sumf[h][:, 0:SB],
                                 sumf[h][:, SB:2 * SB])
            rsq[h] = pool.tile([128, SB], F32, name="rs", tag="rs", bufs=2)
            nc.vector.reciprocal(rsq[h][:, :], stot[:, :])

        def head_sum_post(h):
            # transpose -> [1,S] ds_sum row (PE piece, emitted after a
            # filler so the PE queue has work while the DVE piece resolves)
            hp = 32 * h
            tps = pp.tile([16, 128], F32, name="mmps", tag="mm", bufs=4)
            nc.tensor.transpose(tps[:, :], rsq[h][:, :], ident[:, :])
            st = pool.tile([16, 128], F16, name="st", tag="st", bufs=2)
            nc.vector.tensor_copy(st[:, :], tps[:, :])
            nc.gpsimd.dma_start(ds_sum[hp:hp + 1, :], st[:, :])

        def pair_head(p, units):
            # w = exp(diag) * recip(sumexp); attn = w (bcast) * v, into kh.
            # ACT exps first (no PE coupling), then broadcast+scale chunk
            # groups interleaved with independent PE units.
            for h in (2 * p, 2 * p + 1):
                hp = 32 * h
                nc.scalar.activation(expd[hp:hp + 1, :],
                                     ds_diag[hp:hp + 1, :], AF.Exp)
            if units:
                units.pop(0)()
            for h in (2 * p, 2 * p + 1):
                hp = 32 * h
                nc.vector.tensor_mul(w4[hp:hp + 1, :], expd[hp:hp + 1, :],
                                     ds_sum[hp:hp + 1, :])
            for ck in range(SC):
                for h in (2 * p, 2 * p + 1):
                    hp = 32 * h
                    # K=1 outer-product broadcast of the w row to 128 parts
                    bps = pp.tile([128, 512], F32, name="mmps", tag="mm",
                                  bufs=4)
                    nc.tensor.matmul(bps[:, :], ones1[hp:hp + 1, :],
                                     w4[hp:hp + 1, ck * 512:(ck + 1) * 512],
                                     start=True, stop=True,
                                     tile_position=(hp, 0))
                    # attn scaling straight from psum (no bounce buffer)
                    nc.vector.tensor_mul(kh[h][:, ck * 512:(ck + 1) * 512],
                                         bps[:, :],
                                         vh[h][:, ck * 512:(ck + 1) * 512])
                if units:
                    units.pop(0)()

        def oproj_unit(p, sb, ncx, yts, copy_eng):
            # one 128-row x 512-col chunk of the pair-p output projection
            h0, h1 = 2 * p, 2 * p + 1
            ps = pp.tile([128, 512], F32, name="mmps", tag="mm", bufs=4)
            for i, h in enumerate((h0, h1)):
                nc.tensor.matmul(
                    ps[:, :], kh[h][:, sb * 128:(sb + 1) * 128],
                    wot[:, h, ncx * 512:(ncx + 1) * 512],
                    start=(i == 0), stop=(i == 1))
            dst = yts[:, ncx * 512:(ncx + 1) * 512]
            if copy_eng == "act":
                nc.scalar.activation(dst, ps[:, :], AF.Copy)
            else:
                nc.vector.tensor_copy(dst, ps[:, :])
            if ncx == SC - 1:
                nc.sync.dma_start(y[p, sb * 128:(sb + 1) * 128, :],
                                  yts[:, :])

        # ================= emission =================
        # Phase 0: head-0 K and Q projections in kb-major order - one
        # accumulation step into 6 held PSUM tiles per x block as it lands,
        # so the PE tracks the x DMA stream instead of waiting for all of x.
        kA = pp.tile([128, 1024], F32, name="sps", tag="sco", bufs=2)
        kB = pp.tile([128, 1024], F32, name="sps", tag="sco", bufs=2)
        q4 = [pp.tile([128, 512], F32, name="mmps", tag="mm", bufs=4)
              for _ in range(4)]
        for kb in range(KB):
            st_, sp_ = (kb == 0), (kb == KB - 1)
            for sc in range(2):
                nc.tensor.matmul(kA[:, sc * 512:(sc + 1) * 512],
                                 wkt[:, kb, 0:128],
                                 xsb[kb][:, sc * 512:(sc + 1) * 512],
                                 start=st_, stop=sp_)
            for sc in range(2, 4):
                nc.tensor.matmul(kB[:, (sc - 2) * 512:(sc - 1) * 512],
                                 wkt[:, kb, 0:128],
                                 xsb[kb][:, sc * 512:(sc + 1) * 512],
                                 start=st_, stop=sp_)
            for sc in range(4):
                nc.tensor.matmul(q4[sc][:, :], wqt[:, kb, 0:128],
                                 xsb[kb][:, sc * 512:(sc + 1) * 512],
                                 start=st_, stop=sp_)
        # drain + rope, interleaved with head-1 K projection chunks so the
        # PE has queued work while the DVE/gpsimd rope chain resolves. Q0's
        # mm-pool psums are drained before the K1 chunks rotate onto their
        # banks; the K1 copies are emitted AFTER the rope ops (DVE is
        # in-order, so the reverse would head-of-line block the ropes).
        nc.vector.tensor_copy(kh[0][:, 0:1024], kA[:, :])
        nc.vector.tensor_copy(qh[0][:, 0:512], q4[0][:, :])
        nc.vector.tensor_copy(qh[0][:, 512:1024], q4[1][:, :])
        p0 = proj_mms(wkt, 1, 0)
        rope_half(kh[0], 0)
        nc.vector.tensor_copy(kh[1][:, 0:512], p0[:, :])
        nc.vector.tensor_copy(kh[0][:, 1024:2048], kB[:, :])
        nc.vector.tensor_copy(qh[0][:, 1024:1536], q4[2][:, :])
        nc.vector.tensor_copy(qh[0][:, 1536:2048], q4[3][:, :])
        p1 = proj_mms(wkt, 1, 1)
        rope_half(kh[0], 1)
        nc.vector.tensor_copy(kh[1][:, 512:1024], p1[:, :])
        p2 = proj_mms(wkt, 1, 2)
        rope_half(qh[0], 0)
        nc.vector.tensor_copy(kh[1][:, 1024:1536], p2[:, :])
        p3 = proj_mms(wkt, 1, 3)
        rope_half(qh[0], 1)
        nc.vector.tensor_copy(kh[1][:, 1536:2048], p3[:, :])

        # Filler units: one callable each, emitted between score matmul
        # groups. Order respects cross-engine in-order queues (a DVE/ACT
        # instruction emitted before its producer would deadlock the queue).
        def proj_unit(wt_f, dests, mt, sc, ropes=(), diags=()):
            def f():
                proj_chunk(wt_f(), dests, mt, sc)
                for dst, c in ropes:
                    rope_half(dst, c)
                for h, c in diags:
                    diag_half(h, c)
            return f

        wvt = None
        wot = None

        def load_wv():
            nonlocal wvt
            wvt = pool.tile([128, KB, 512], F16, name="wt", tag="w", bufs=2)
            nc.sync.dma_start(wvt[:, :, :], wv_p[:, :, :])

        def load_wo():
            nonlocal wot
            wot = pool.tile([128, HPC, S], F16, name="wt", tag="w", bufs=2)
            nc.sync.dma_start(wot[:, :, :], wo_p[:, :, :])

        def head_units(mt):
            # 8 units: K chunks (rope halves attached after chunks 1/3),
            # then Q chunks (+diag once both q and k halves are roped).
            units = []
            for sc in range(SC):
                ropes = [(kh[mt], 0)] if sc == 1 else \
                        [(kh[mt], 1)] if sc == 3 else ()
                units.append(proj_unit(lambda: wkt, kh, mt, sc, ropes=ropes))
            for sc in range(SC):
                ropes = [(qh[mt], 0)] if sc == 1 else \
                        [(qh[mt], 1)] if sc == 3 else ()
                diags = [(mt, 0)] if sc == 1 else \
                        [(mt, 1)] if sc == 3 else ()
                units.append(proj_unit(lambda: wqt, qh, mt, sc, ropes=ropes,
                                       diags=diags))
            return units

        def v_units(mt):
            return [proj_unit(lambda: wvt, vh, mt, sc) for sc in range(SC)]

        def diag0_unit():
            # deferred prologue pieces: head-1 K ropes + head-0 diag
            def f():
                rope_half(kh[1], 0)
                rope_half(kh[1], 1)
                diag_half(0, 0)
                diag_half(0, 1)
            return f

        def q_units(mt):
            units = [proj_unit(lambda: wqt, qh, mt, 0)]
            units.append(proj_unit(lambda: wqt, qh, mt, 1,
                                   ropes=[(qh[mt], 0)], diags=[(mt, 0)]))
            units.append(proj_unit(lambda: wqt, qh, mt, 2))
            units.append(proj_unit(lambda: wqt, qh, mt, 3,
                                   ropes=[(qh[mt], 1)], diags=[(mt, 1)]))
            return units

        # h0 stream fillers: deferred head-0 diag + k1 ropes, Q1, K2/Q2,
        # and the first 3 K3 chunks
        k3q3 = head_units(3)
        fill_h0 = ([diag0_unit()] + q_units(1) + head_units(2) + k3q3[:3])
        # h1: last K3 chunk, wv load (overlaps Q3), Q3, wo load, V heads 0,1
        fill_h1 = ([k3q3[3], load_wv] + k3q3[4:] + [load_wo]
                   + v_units(0) + v_units(1))
        # h2: V heads 2,3 (their copies are needed only by pair_head(1))
        fill_h2 = v_units(2) + v_units(3)

        yts = {}
        ofill = [(0, sb, ncx) for sb in range(SB) for ncx in range(SC)]

        def oproj_pop(n, copy_eng="dve", keep=0):
            for _ in range(n):
                if len(ofill) <= keep:
                    return
                p, sb, ncx = ofill.pop(0)
                if ncx == 0:
                    yts[p] = pool.tile([128, S], F16, name="yt",
                                       tag="yt", bufs=2)
                oproj_unit(p, sb, ncx, yts[p], copy_eng)

        def stream(h, fill, per_sq_oproj=0):
            # scores BEFORE the slot's fillers: Bacc lowers cross-engine
            # deps as monotonic queue-count gates, so an exp emitted after
            # a filler would wait for that filler's DVE copy too.
            for sq in range(SB):
                scores_half(h, sq, 0)
                scores_half(h, sq, 1)
                if fill:
                    fill.pop(0)()
                if len(fill) > SB - 1 - sq:
                    fill.pop(0)()
                if per_sq_oproj:
                    # hold back 5 pair-0 units to feed the PE through the
                    # pair-1 boundary chain
                    oproj_pop(per_sq_oproj, keep=5)
            head_sum_pre(h)

        stream(0, fill_h0)
        fill_h1.pop(0)()                       # K3c0: PE work for the gap
        head_sum_post(0)
        stream(1, fill_h1)
        fill_h2.pop(0)()                       # V2c0
        head_sum_post(1)
        pair_head(0, fill_h2)                  # interleaves V2c1..V3c1
        stream(2, fill_h2, per_sq_oproj=2)     # V3c2,V3c3 + pair-0 oproj
        oproj_pop(1, keep=5)
        head_sum_post(2)
        stream(3, [], per_sq_oproj=2)

        def reserve_unit():
            def f():
                oproj_pop(1)
            return f

        head_sum_post(3)
        pair_head(1, [reserve_unit() for _ in range(5)])

        # tail: pair-1 output projection; psum->sbuf copies alternate
        # DVE/ACT (ACT is idle by now).
        ofill += [(1, sb, ncx) for sb in range(SB) for ncx in range(SC)]
        i = 0
        while ofill:
            oproj_pop(1, "act" if i % 2 else "dve")
            i += 1

    nc.compile()
    return nc


def _get_nc():
    if "nc" not in _CACHE:
        _CACHE["nc"] = _build_nc()
    return _CACHE["nc"]


_PERM = np.concatenate([np.arange(0, DH, 2), np.arange(1, DH, 2)])


def _host_inputs(x, rope_cos, rope_sin, Wq, Wk, Wv, Wo):
    """Build the 8 per-core input maps."""
    f16 = np.float16
    cosT = np.ascontiguousarray(np.asarray(rope_cos, np.float32)[0, :, 0, :].T)
    sinT = np.ascontiguousarray(np.asarray(rope_sin, np.float32)[0, :, 0, :].T)
    ra = np.concatenate([cosT, cosT], 0).astype(f16)
    rb = np.concatenate([-sinT, sinT], 0).astype(f16)

    Wq = np.asarray(Wq, np.float32)
    Wk = np.asarray(Wk, np.float32)
    Wv = np.asarray(Wv, np.float32)
    Wo = np.asarray(Wo, np.float32)
    x = np.asarray(x, np.float32)

    xTb = [np.ascontiguousarray(x[b].T).astype(f16) for b in range(B)]
    scale = DH ** -0.5

    in_maps = []
    for core in range(NCORES):
        b, g = divmod(core, HPC)
        hs = g * HPC
        rows = np.concatenate(
            [h * DH + _PERM for h in range(hs, hs + HPC)])      # deinterleave
        rows_v = np.arange(hs * DH, (hs + HPC) * DH)
        in_maps.append({
            "xT": xTb[b],
            "wq": np.ascontiguousarray((Wq[rows] * scale).T).astype(f16),
            "wk": np.ascontiguousarray(Wk[rows].T).astype(f16),
            "wv": np.ascontiguousarray(Wv[rows_v].T).astype(f16),
            "wo": np.ascontiguousarray(Wo[:, rows_v].T).astype(f16),
            "ropeA": ra,
            "ropeB": rb,
        })
    return in_maps


def kernel(x, rope_cos, rope_sin, Wq, Wk, Wv, Wo, _trace=False, _trace_cores=None):
    from concourse.bass_utils import run_bass_kernel_spmd

    nc = _get_nc()
    in_maps = _host_inputs(x, rope_cos, rope_sin, Wq, Wk, Wv, Wo)
    res = run_bass_kernel_spmd(nc, in_maps, list(range(NCORES)),
                               trace=_trace, trace_cores=_trace_cores)
    _CACHE["last_result"] = res

    out = np.zeros((B, S, D), np.float32)
    for core in range(NCORES):
        b = core // HPC
        out[b] += res.results[core]["y"].astype(np.float32).sum(axis=0)
    return out


# revision 31
# speedup vs baseline: 1.3017x; 1.0474x over previous
"""Trainium2 Bass kernel for the MHA-with-diagonal-softmax module.

Computation (per batch b):
    q = rope(x @ Wq.T), k = rope(x @ Wk.T), v = x @ Wv.T      (per head, DH=128)
    sumexp[s,h] = sum_k exp(q_h[s] . k_h[k] * DH^-0.5)
    diag[s,h]   = q_h[s] . k_h[s] * DH^-0.5
    w = exp(diag) / sumexp
    out = (w * v) @ Wo.T

Sharding: 8 cores = 2 (batch) x 4 (head groups of 4 heads).
Each core computes q/k/v for its 4 heads in transposed [head_dim, seq]
layout, the per-position softmax-diagonal weights, and a partial output
projection (its heads' rows of Wo), written as 2 head-pair partials that
the host sums.

Schedule (v2): the exp(scores) stream on the ACT engine is the second-
largest engine load (~180us) and is started as early as possible (~32us,
right after head 0's K/Q projections + rope). All remaining PE work
(K/Q proj heads 1-3, V proj, output proj of pair 0) is emitted as
fine-grained filler between score matmuls so the PE queue never stalls
(in-order queues: a waiting instruction blocks everything behind it) and
the PE stays at the 2.4GHz p-state. Scores PSUM is a rotating 3-buffer
of [128,1024] halves (6 banks) + a 2-buffer [128,512] pool (2 banks) for
everything else, so score matmuls never wait on exp drain.

On-chip dtype is fp16 (same PE throughput as bf16, 8x lower rounding
error - matters because exp() amplifies absolute score error), with fp32
PSUM accumulation everywhere.
"""

import numpy as np
from contextlib import ExitStack

# Problem constants (hardcoded per harness contract).
B, S, D, H, DH = 2, 2048, 2048, 16, 128
HPC = 4            # heads per core
NHL = HPC * DH     # 512 local head dims per core
KB = D // 128      # 16 contraction blocks
SB = S // 128      # 16 seq blocks of 128
SC = S // 512      # 4 seq/emb chunks of 512
NCORES = 8

_CACHE = {}


def _build_nc():
    import concourse.bass as bass
    import concourse.tile as tile
    from concourse import bacc, mybir
    from concourse.masks import make_identity

    F16 = mybir.dt.float16
    F32 = mybir.dt.float32
    AF = mybir.ActivationFunctionType

    # Bacc (not raw Bass): its compile() splits multi-sem waits into
    # event-semaphore instructions - HW allows at most 1 wait per inst.
    nc = bacc.Bacc("TRN2", target_bir_lowering=False, debug=False)

    # weights arrive pre-arranged partition-major on the host so each DMA
    # is 128 x 8KB contiguous descriptors (1KB-row descriptors measured
    # ~120GB/s and hog the 4-deep DMA rings)
    xT = nc.dram_tensor("xT", [D, S], F16, kind="ExternalInput").ap()
    wq = nc.dram_tensor("wq", [128, KB * 512], F16, kind="ExternalInput").ap()
    wk = nc.dram_tensor("wk", [128, KB * 512], F16, kind="ExternalInput").ap()
    wv = nc.dram_tensor("wv", [128, KB * 512], F16, kind="ExternalInput").ap()
    wo = nc.dram_tensor("wo", [128, HPC * S], F16, kind="ExternalInput").ap()
    ropeA = nc.dram_tensor("ropeA", [128, S], F16, kind="ExternalInput").ap()
    ropeB = nc.dram_tensor("ropeB", [128, S], F16, kind="ExternalInput").ap()
    y = nc.dram_tensor("y", [2, S, D], F16, kind="ExternalOutput").ap()

    xT_r = xT.rearrange("(a p) s -> a p s", p=128)
    wq_p = wq.rearrange("p (a m) -> p a m", a=KB)
    wk_p = wk.rearrange("p (a m) -> p a m", a=KB)
    wv_p = wv.rearrange("p (a m) -> p a m", a=KB)
    wo_p = wo.rearrange("p (h n) -> p h n", h=HPC)

    with tile.TileContext(nc) as tc, ExitStack() as ctx:
        pool = ctx.enter_context(tc.tile_pool(name="sb", bufs=1))
        pp = ctx.enter_context(tc.tile_pool(name="ps", bufs=1, space="PSUM"))

        # ---- constants (gpsimd SWDGE: small, keeps HWDGE queues for x/w) --
        ra = pool.tile([128, S], F16, name="ra")
        rb = pool.tile([128, S], F16, name="rb")
        nc.gpsimd.dma_start(ra[:, :], ropeA[:, :])
        nc.gpsimd.dma_start(rb[:, :], ropeB[:, :])
        ident = pool.tile([128, 128], F32, name="ident")
        make_identity(nc, ident[:, :])
        ones1 = pool.tile([128, 128], F16, name="ones1")
        nc.gpsimd.memset(ones1[:, :], 1.0)

        # ---- big input DMAs ----
        # wk/wq first (small, unblock the PE), then x one block per kb so
        # the head-0 K/Q projections can accumulate into held PSUM tiles as
        # each block lands. One TILE per kb block: the Tile framework tracks
        # dependencies per tile, so a single xsb tile would make the first
        # matmul wait for all 16 DMAs. Issues alternate SP/ACT queues.
        xsb = [pool.tile([128, S], F16, name=f"xsb{kb}") for kb in range(KB)]
        wkt = pool.tile([128, KB, 512], F16, name="wt", tag="w", bufs=2)
        wqt = pool.tile([128, KB, 512], F16, name="wt", tag="w", bufs=2)
        nc.sync.dma_start(wkt[:, :, :], wk_p[:, :, :])
        nc.scalar.dma_start(wqt[:, :, :], wq_p[:, :, :])
        engs = [nc.sync, nc.scalar, nc.gpsimd]
        for kb in range(KB):
            engs[kb % 3].dma_start(xsb[kb][:, :], xT_r[kb])

        # ---- persistent q/k/v head tiles ([head_dim, seq] layout) ----
        qh = [pool.tile([128, S], F16, name=f"qh{h}") for h in range(HPC)]
        kh = [pool.tile([128, S], F16, name=f"kh{h}") for h in range(HPC)]
        vh = [pool.tile([128, S], F16, name=f"vh{h}") for h in range(HPC)]

        # per-head row vectors live at partition 32*h (engine ops only
        # support start partitions that are multiples of 32)
        ds_diag = pool.tile([128, S], F32, name="ds_diag")
        ds_sum = pool.tile([128, S], F16, name="ds_sum")
        w4 = pool.tile([128, S], F16, name="w4")
        expd = pool.tile([128, S], F16, name="expd")
        # 2 accum columns per sq block (exp is done in 1024-wide halves)
        sumf = [pool.tile([128, 2 * SB], F32, name=f"sumf{h}")
                for h in range(HPC)]

        def proj_mms(wt, mt, sc):
            # (wt[:, :, mt] block).T @ x chunk -> a rotating psum tile
            ps = pp.tile([128, 512], F32, name="mmps", tag="mm", bufs=4)
            for kb in range(KB):
                nc.tensor.matmul(
                    ps[:, :],
                    wt[:, kb, mt * 128:(mt + 1) * 128],
                    xsb[kb][:, sc * 512:(sc + 1) * 512],
                    start=(kb == 0), stop=(kb == KB - 1))
            return ps

        def proj_chunk(wt, dests, mt, sc):
            # psum->sbuf copy on DVE (ACT is reserved for the exp stream).
            ps = proj_mms(wt, mt, sc)
            nc.vector.tensor_copy(
                dests[mt][:, sc * 512:(sc + 1) * 512], ps[:, :])

        def rope_half(dst, c):
            # dst half (in place): top = te*cos - to*sin ; bot = te*sin+to*cos
            # ra = [cosT; cosT], rb = [-sinT; sinT]; swap = halves exchanged.
            sl = slice(c * 1024, (c + 1) * 1024)
            # SWDGE (gpsimd) keeps this 1 queue -> 1 sem; a wide HWDGE
            # sbuf->sbuf DMA fans out over many queues and blows the
            # consumer's sync-wait slot budget.
            swp = pool.tile([128, 1024], F16, name="swp", tag="swp", bufs=2)
            nc.gpsimd.dma_start(swp[0:64, :], dst[64:128, sl])
            nc.gpsimd.dma_start(swp[64:128, :], dst[0:64, sl])
            u = pool.tile([128, 1024], F16, name="u", tag="sc", bufs=2)
            nc.vector.tensor_mul(u[:, :], dst[:, sl], ra[:, sl])
            v2 = pool.tile([128, 1024], F16, name="v2", tag="sc", bufs=2)
            nc.vector.tensor_mul(v2[:, :], swp[:, :], rb[:, sl])
            nc.vector.tensor_add(dst[:, sl], u[:, :], v2[:, :])

        def diag_half(h, c):
            # ds_diag[32h, s-half] = sum_m qh[h][m,s] * kh[h][m,s]
            hp = 32 * h
            sl = slice(c * 1024, (c + 1) * 1024)
            pr = pool.tile([128, 1024], F16, name="pr", tag="pr", bufs=2)
            nc.vector.tensor_mul(pr[:, :], qh[h][:, sl], kh[h][:, sl])
            for cc in range(2):
                dps = pp.tile([128, 512], F32, name="mmps", tag="mm", bufs=4)
                nc.tensor.matmul(dps[:, :], ones1[:, :],
                                 pr[:, cc * 512:(cc + 1) * 512],
                                 start=True, stop=True)
                o = (2 * c + cc) * 512
                nc.vector.tensor_copy(ds_diag[hp:hp + 1, o:o + 512],
                                      dps[hp:hp + 1, :])

        # ====== scores stream ======
        ex = pool.tile([128, 1024], F16, name="ex")

        def scores_half(h, sq, half):
            # 2 score MMs into a rotating [128,1024] psum half, one exp
            # with fused row-sum into sumf[h][:, half*SB+sq].
            sps = pp.tile([128, 1024], F32, name="sps", tag="sco", bufs=2)
            for cc in range(2):
                ck = 2 * half + cc
                nc.tensor.matmul(sps[:, cc * 512:(cc + 1) * 512],
                                 qh[h][:, sq * 128:(sq + 1) * 128],
                                 kh[h][:, ck * 512:(ck + 1) * 512],
                                 start=True, stop=True)
            col = half * SB + sq
            nc.scalar.activation(ex[:, :], sps[:, :], AF.Exp,
                                 accum_out=sumf[h][:, col:col + 1])

        rsq = {}

        def head_sum_pre(h):
            # DVE-only piece: sum the 2 half-accums, reciprocal
            stot = pool.tile([128, SB], F32, name="stot", tag="rs", bufs=2)
            nc.vector.tensor_add(stot[:, :], sumf[h][:, 0:SB],
                                 sumf[h][:, SB:2 * SB])
            rsq[h] = pool.tile([128, SB], F32, name="rs", tag="rs", bufs=2)
            nc.vector.reciprocal(rsq[h][:, :], stot[:, :])

        def head_sum_post(h):
            # transpose -> [1,S] ds_sum row (PE piece, emitted after a
            # filler so the PE queue has work while the DVE piece resolves)
            hp = 32 * h
            tps = pp.tile([16, 128], F32, name="mmps", tag="mm", bufs=4)
            nc.tensor.transpose(tps[:, :], rsq[h][:, :], ident[:, :])
            st = pool.tile([16, 128], F16, name="st", tag="st", bufs=2)
            nc.vector.tensor_copy(st[:, :], tps[:, :])
            nc.gpsimd.dma_start(ds_sum[hp:hp + 1, :], st[:, :])

        def pair_head(p, units):
            # w = exp(diag) * recip(sumexp); attn = w (bcast) * v, into kh.
            # ACT exps first (no PE coupling), then broadcast+scale chunk
            # groups interleaved with independent PE units.
            for h in (2 * p, 2 * p + 1):
                hp = 32 * h
                nc.scalar.activation(expd[hp:hp + 1, :],
                                     ds_diag[hp:hp + 1, :], AF.Exp)
            if units:
                units.pop(0)()
            for h in (2 * p, 2 * p + 1):
                hp = 32 * h
                nc.vector.tensor_mul(w4[hp:hp + 1, :], expd[hp:hp + 1, :],
                                     ds_sum[hp:hp + 1, :])
            for ck in range(SC):
                for h in (2 * p, 2 * p + 1):
                    hp = 32 * h
                    # K=1 outer-product broadcast of the w row to 128 parts
                    bps = pp.tile([128, 512], F32, name="mmps", tag="mm",
                                  bufs=4)
                    nc.tensor.matmul(bps[:, :], ones1[hp:hp + 1, :],
                                     w4[hp:hp + 1, ck * 512:(ck + 1) * 512],
                                     start=True, stop=True,
                                     tile_position=(hp, 0))
                    # attn scaling straight from psum (no bounce buffer)
                    nc.vector.tensor_mul(kh[h][:, ck * 512:(ck + 1) * 512],
                                         bps[:, :],
                                         vh[h][:, ck * 512:(ck + 1) * 512])
                if units:
                    units.pop(0)()

        def oproj_unit(p, sb, ncx, yts, copy_eng):
            # one 128-row x 512-col chunk of the pair-p output projection
            h0, h1 = 2 * p, 2 * p + 1
            ps = pp.tile([128, 512], F32, name="mmps", tag="mm", bufs=4)
            for i, h in enumerate((h0, h1)):
                nc.tensor.matmul(
                    ps[:, :], kh[h][:, sb * 128:(sb + 1) * 128],
                    wot[:, h, ncx * 512:(ncx + 1) * 512],
                    start=(i == 0), stop=(i == 1))
            dst = yts[:, ncx * 512:(ncx + 1) * 512]
            if copy_eng == "act":
                nc.scalar.activation(dst, ps[:, :], AF.Copy)
            else:
                nc.vector.tensor_copy(dst, ps[:, :])
            if ncx == SC - 1:
                nc.sync.dma_start(y[p, sb * 128:(sb + 1) * 128, :],
                                  yts[:, :])

        # ================= emission =================
        # Phase 0: head-0 K and Q projections in kb-major order - one
        # accumulation step into 6 held PSUM tiles per x block as it lands,
        # so the PE tracks the x DMA stream instead of waiting for all of x.
        kA = pp.tile([128, 1024], F32, name="sps", tag="sco", bufs=2)
        kB = pp.tile([128, 1024], F32, name="sps", tag="sco", bufs=2)
        q4 = [pp.tile([128, 512], F32, name="mmps", tag="mm", bufs=4)
              for _ in range(4)]
        for kb in range(KB):
            st_, sp_ = (kb == 0), (kb == KB - 1)
            for sc in range(2):
                nc.tensor.matmul(kA[:, sc * 512:(sc + 1) * 512],
                                 wkt[:, kb, 0:128],
                                 xsb[kb][:, sc * 512:(sc + 1) * 512],
                                 start=st_, stop=sp_)
            for sc in range(2, 4):
                nc.tensor.matmul(kB[:, (sc - 2) * 512:(sc - 1) * 512],
                                 wkt[:, kb, 0:128],
                                 xsb[kb][:, sc * 512:(sc + 1) * 512],
                                 start=st_, stop=sp_)
            for sc in range(4):
                nc.tensor.matmul(q4[sc][:, :], wqt[:, kb, 0:128],
                                 xsb[kb][:, sc * 512:(sc + 1) * 512],
                                 start=st_, stop=sp_)
        # drain + rope, interleaved with head-1 K projection chunks so the
        # PE has queued work while the DVE/gpsimd rope chain resolves. Q0's
        # mm-pool psums are drained before the K1 chunks rotate onto their
        # banks; the K1 copies are emitted AFTER the rope ops (DVE is
        # in-order, so the reverse would head-of-line block the ropes).
        nc.vector.tensor_copy(kh[0][:, 0:1024], kA[:, :])
        nc.vector.tensor_copy(qh[0][:, 0:512], q4[0][:, :])
        nc.vector.tensor_copy(qh[0][:, 512:1024], q4[1][:, :])
        p0 = proj_mms(wkt, 1, 0)
        rope_half(kh[0], 0)
        nc.vector.tensor_copy(kh[1][:, 0:512], p0[:, :])
        nc.vector.tensor_copy(kh[0][:, 1024:2048], kB[:, :])
        nc.vector.tensor_copy(qh[0][:, 1024:1536], q4[2][:, :])
        nc.vector.tensor_copy(qh[0][:, 1536:2048], q4[3][:, :])
        p1 = proj_mms(wkt, 1, 1)
        rope_half(kh[0], 1)
        nc.vector.tensor_copy(kh[1][:, 512:1024], p1[:, :])
        p2 = proj_mms(wkt, 1, 2)
        rope_half(qh[0], 0)
        nc.vector.tensor_copy(kh[1][:, 1024:1536], p2[:, :])
        p3 = proj_mms(wkt, 1, 3)
        rope_half(qh[0], 1)
        nc.vector.tensor_copy(kh[1][:, 1536:2048], p3[:, :])

        # Filler micro-units (~1.7us of PE each), emitted between score
        # matmul groups. Small units distribute evenly into the ~2us of PE
        # slack per score block; a monolithic 3.5us chunk can't. Each proj
        # chunk is two halves sharing one psum tile (held across the gap);
        # ropes/diags are standalone units. Order respects cross-engine
        # in-order queues (an instruction emitted before its producer
        # would head-of-line block its engine).
        def chunk_units(wt_f, dests, mt, sc):
            cell = []

            def fa():
                ps = pp.tile([128, 512], F32, name="mmps", tag="mm", bufs=4)
                cell.append(ps)
                for kb in range(KB // 2):
                    nc.tensor.matmul(
                        ps[:, :], wt_f()[:, kb, mt * 128:(mt + 1) * 128],
                        xsb[kb][:, sc * 512:(sc + 1) * 512],
                        start=(kb == 0), stop=False)

            def fb():
                ps = cell[0]
                for kb in range(KB // 2, KB):
                    nc.tensor.matmul(
                        ps[:, :], wt_f()[:, kb, mt * 128:(mt + 1) * 128],
                        xsb[kb][:, sc * 512:(sc + 1) * 512],
                        start=False, stop=(kb == KB - 1))
                nc.vector.tensor_copy(
                    dests[mt][:, sc * 512:(sc + 1) * 512], ps[:, :])
            return [fa, fb]

        def rope_unit(dst, c):
            return [lambda: rope_half(dst, c)]

        def diag_units(h):
            return [lambda: diag_half(h, 0), lambda: diag_half(h, 1)]

        wvt = None
        wot = None

        def load_wv():
            nonlocal wvt
            wvt = pool.tile([128, KB, 512], F16, name="wt", tag="w", bufs=2)
            nc.sync.dma_start(wvt[:, :, :], wv_p[:, :, :])

        def load_wo():
            nonlocal wot
            wot = pool.tile([128, HPC, S], F16, name="wt", tag="w", bufs=2)
            nc.sync.dma_start(wot[:, :, :], wo_p[:, :, :])

        def head_units(mt):
            # micro-units for one head's K then Q projections, with rope
            # halves as soon as their chunks land and diags after the ropes
            units = []
            for sc in range(SC):
                units += chunk_units(lambda: wkt, kh, mt, sc)
                if sc == 1:
                    units += rope_unit(kh[mt], 0)
                if sc == 3:
                    units += rope_unit(kh[mt], 1)
            for sc in range(SC):
                units += chunk_units(lambda: wqt, qh, mt, sc)
                if sc == 1:
                    units += rope_unit(qh[mt], 0)
                    units.append(lambda mt=mt: diag_half(mt, 0))
                if sc == 3:
                    units += rope_unit(qh[mt], 1)
                    units.append(lambda mt=mt: diag_half(mt, 1))
            return units

        def v_units(mt):
            units = []
            for sc in range(SC):
                units += chunk_units(lambda: wvt, vh, mt, sc)
            return units

        def k_units(mt):
            units = []
            for sc in range(SC):
                units += chunk_units(lambda: wkt, kh, mt, sc)
                if sc == 1:
                    units += rope_unit(kh[mt], 0)
                if sc == 3:
                    units += rope_unit(kh[mt], 1)
            return units

        def q_units(mt):
            units = []
            for sc in range(SC):
                units += chunk_units(lambda: wqt, qh, mt, sc)
                if sc == 1:
                    units += rope_unit(qh[mt], 0)
                    units.append(lambda mt=mt: diag_half(mt, 0))
                if sc == 3:
                    units += rope_unit(qh[mt], 1)
                    units.append(lambda mt=mt: diag_half(mt, 1))
            return units

        # one flat micro-unit list consumed across the h0+h1 streams with
        # proportional pacing (avoids fat early slots + starved late slots)
        fill_a = ([lambda: diag_half(0, 0), lambda: diag_half(0, 1)]
                  + rope_unit(kh[1], 0) + rope_unit(kh[1], 1)
                  + q_units(1) + k_units(2) + q_units(2) + k_units(3)
                  + [load_wv] + q_units(3) + [load_wo]
                  + v_units(0) + v_units(1))

        yts = {}
        ofill = [(0, sb, ncx) for sb in range(SB) for ncx in range(SC)]

        def oproj_pop(n, copy_eng="dve", keep=0):
            for _ in range(n):
                if len(ofill) <= keep:
                    return
                p, sb, ncx = ofill.pop(0)
                if ncx == 0:
                    yts[p] = pool.tile([128, S], F16, name="yt",
                                       tag="yt", bufs=3)
                oproj_unit(p, sb, ncx, yts[p], copy_eng)

        def stream(h, fill, frac=1.0, per_sq_oproj=0):
            # scores BEFORE the slot's fillers: Bacc lowers cross-engine
            # deps as monotonic queue-count gates, so an exp emitted after
            # a filler would wait for that filler's DVE copy too.
            take = int(round(len(fill) * frac))
            taken = 0
            for sq in range(SB):
                scores_half(h, sq, 0)
                scores_half(h, sq, 1)
                tgt = take * (sq + 1) // SB
                while taken < tgt and fill:
                    fill.pop(0)()
                    taken += 1
                if per_sq_oproj:
                    # hold back 5 pair-0 units to feed the PE through the
                    # pair-1 boundary chain
                    oproj_pop(per_sq_oproj, keep=5)
            head_sum_pre(h)

        stream(0, fill_a, frac=0.5)
        fill_a.pop(0)()
        head_sum_post(0)
        stream(1, fill_a)
        fill_b = v_units(2) + v_units(3)
        fill_b.pop(0)()
        head_sum_post(1)
        pair_head(0, fill_b)
        stream(2, fill_b, per_sq_oproj=2)
        oproj_pop(1, keep=5)
        head_sum_post(2)
        stream(3, [], per_sq_oproj=2)

        def reserve_unit():
            def f():
                oproj_pop(1)
            return f

        head_sum_post(3)
        pair_head(1, [reserve_unit() for _ in range(5)])

        # tail: pair-1 output projection; psum->sbuf copies alternate
        # DVE/ACT (ACT is idle by now).
        ofill += [(1, sb, ncx) for sb in range(SB) for ncx in range(SC)]
        i = 0
        while ofill:
            oproj_pop(1, "act" if i % 2 else "dve")
            i += 1

    nc.compile()
    return nc


def _get_nc():
    if "nc" not in _CACHE:
        _CACHE["nc"] = _build_nc()
    return _CACHE["nc"]


_PERM = np.concatenate([np.arange(0, DH, 2), np.arange(1, DH, 2)])


def _host_inputs(x, rope_cos, rope_sin, Wq, Wk, Wv, Wo):
    """Build the 8 per-core input maps."""
    f16 = np.float16
    cosT = np.ascontiguousarray(np.asarray(rope_cos, np.float32)[0, :, 0, :].T)
    sinT = np.ascontiguousarray(np.asarray(rope_sin, np.float32)[0, :, 0, :].T)
    ra = np.concatenate([cosT, cosT], 0).astype(f16)
    rb = np.concatenate([-sinT, sinT], 0).astype(f16)

    Wq = np.asarray(Wq, np.float32)
    Wk = np.asarray(Wk, np.float32)
    Wv = np.asarray(Wv, np.float32)
    Wo = np.asarray(Wo, np.float32)
    x = np.asarray(x, np.float32)

    xTb = [np.ascontiguousarray(x[b].T).astype(f16) for b in range(B)]
    scale = DH ** -0.5

    def pm(arr, nblk):
        # partition-major DMA layout: [p, blk*inner + m] = arr[blk*128+p, m]
        inner = arr.shape[1]
        return np.ascontiguousarray(
            arr.reshape(nblk, 128, inner).transpose(1, 0, 2)
            .reshape(128, nblk * inner))

    in_maps = []
    for core in range(NCORES):
        b, g = divmod(core, HPC)
        hs = g * HPC
        rows = np.concatenate(
            [h * DH + _PERM for h in range(hs, hs + HPC)])      # deinterleave
        rows_v = np.arange(hs * DH, (hs + HPC) * DH)
        in_maps.append({
            "xT": xTb[b],
            "wq": pm((Wq[rows] * scale).T, KB).astype(f16),
            "wk": pm(Wk[rows].T, KB).astype(f16),
            "wv": pm(Wv[rows_v].T, KB).astype(f16),
            "wo": pm(Wo[:, rows_v].T, HPC).astype(f16),
            "ropeA": ra,
            "ropeB": rb,
        })
    return in_maps


def kernel(x, rope_cos, rope_sin, Wq, Wk, Wv, Wo, _trace=False, _trace_cores=None):
    from concourse.bass_utils import run_bass_kernel_spmd

    nc = _get_nc()
    in_maps = _host_inputs(x, rope_cos, rope_sin, Wq, Wk, Wv, Wo)
    res = run_bass_kernel_spmd(nc, in_maps, list(range(NCORES)),
                               trace=_trace, trace_cores=_trace_cores)
    _CACHE["last_result"] = res

    out = np.zeros((B, S, D), np.float32)
    for core in range(NCORES):
        b = core // HPC
        out[b] += res.results[core]["y"].astype(np.float32).sum(axis=0)
    return out
